# revision 18
# baseline (speedup 1.0000x reference)
"""Trainium2 Bass kernel for the DeltaHebbian (gated delta-rule) block.

Sharding: 8 cores = 4 batches x 2 head-groups (4 heads each). Each core gets
its batch's x with columns rotated so its head-group occupies cols 0:512, and
computes partial_out.T = (gated_o @ W_out_slice.T).T.  Host sums the two
partials per batch and adds x.

Per-core algorithm (chunked delta rule, CHUNK=64):
  phase 1 (token-parallel): projections, key normalization, per-chunk decay
  cumsums, masked key-product matrices M / M.T / attn.T, and the UT-transform
  inverse A.T = ((I+M)^-1).T via the telescoping factorization
  (I-M)(I+M^2)(I+M^4)(I+M^8)  (exact to ~4e-5 on this data: |M^16| ~ 5e-5).
  phase 2 (sequential over chunks, 4 heads interleaved): the state recurrence.
"""

import sys

for _p in ("/opt/trn_rl_repo",):
    if _p not in sys.path:
        sys.path.append(_p)

from contextlib import ExitStack

import numpy as np
import ml_dtypes

import concourse.bass as bass
import concourse.mybir as mybir
import concourse.tile as tile

F32 = mybir.dt.float32
BF16 = mybir.dt.bfloat16
OP = mybir.AluOpType
AF = mybir.ActivationFunctionType

# problem constants
B, T, D = 4, 8192, 1024
HD = 128          # head dim
C = 64            # chunk length
HG = 4            # heads per core
GC = HG * HD      # 512 group columns
NCORES = 8
NQ = 12           # bundle quantities per head
# bundle column indices (per head, stride NQ); cols 0..3 are the bf16
# plane factors (transposed then token-broadcast)
(QRA, QFB, QWB, QG, QF, QFSH, QEDEC, QEDECI, QDEC, QEDEC2, QBETA,
 QFDW) = range(12)
NBF = 4           # bf16 transposed rows per head: cols 0..3


def _consts():
    ii = np.arange(128)
    jj = np.arange(512)
    pi = ii[:, None] % 64
    qi = jj[None, :] % 64
    c = {}
    c["i2x8"] = (pi == qi).astype(np.float32)
    c["mSL"] = (pi > qi).astype(np.float32)      # keep i>j   (M)
    c["mSU"] = (qi > pi).astype(np.float32)      # keep j>i   (M.T)
    c["mUI"] = (qi >= pi).astype(np.float32)     # keep i>=j  (attn.T)
    k = np.arange(128)
    m = np.arange(128)
    same = (k[:, None] // 64) == (m[None, :] // 64)
    c["triucum"] = (same & ((k[:, None] % 64) <= (m[None, :] % 64))).astype(np.float32)
    c["e64sel"] = (k[:, None] == (m[None, :] // 64) * 64 + 63).astype(np.float32)
    c["identbf"] = np.eye(128).astype(ml_dtypes.bfloat16)
    c["identf"] = np.eye(128).astype(np.float32)
    c["ones4"] = np.ones((128, 4), np.float32)
    sh1 = (k[:, None] == m[None, :] - 1).astype(np.float32)   # out[m]=in[m-1]
    c["sh1f"] = sh1
    c["sh1bf"] = sh1.astype(ml_dtypes.bfloat16)
    s127 = np.zeros((128, 128), np.float32)   # out row0 = in row127, rest += 0
    s127[127, 0] = 1.0
    c["sel127f"] = s127
    c["sel127bf"] = s127.astype(ml_dtypes.bfloat16)
    # bf16 row selectors: target t -> (16, 128) block with row t all-ones
    selbf = np.zeros((16, 16 * 128), np.float32)
    for t in range(16):
        selbf[t, t * 128:(t + 1) * 128] = 1.0
    c["selbf"] = selbf.astype(ml_dtypes.bfloat16)
    sel2 = np.zeros((128, 2 * 128), np.float32)  # [dec-sel | edec-sel] at rows 0/64
    for hh in (0, 64):
        sel2[hh + 0, 0:128] = 1.0
        sel2[hh + 1, 128:256] = 1.0
    c["sel2f"] = sel2
    return c


def build_nc(Ttot=T, TSEG=512, stage=5):
    assert Ttot % TSEG == 0 and TSEG == 512
    NSEG = Ttot // TSEG
    NTILE = TSEG // 128
    NCHS = TSEG // C

    nc = bass.Bass()
    xth = nc.dram_tensor("xth", (8, 128, Ttot + 1), BF16, kind="ExternalInput")
    xnh = nc.dram_tensor("xnh", (Ttot + 1, GC), BF16, kind="ExternalInput")
    wcat = nc.dram_tensor("wcat", (128, 8, GC), BF16, kind="ExternalInput")
    wsml = nc.dram_tensor("wsml", (128, 8, 12), BF16, kind="ExternalInput")
    wout = nc.dram_tensor("wout", (128, HG, 1024), BF16, kind="ExternalInput")
    dtb = nc.dram_tensor("dtb", (128, 4), F32, kind="ExternalInput")
    aneg = nc.dram_tensor("aneg", (128, 4), F32, kind="ExternalInput")
    outp = nc.dram_tensor("outp", (8, 128, Ttot), BF16, kind="ExternalOutput")

    cst = _consts()
    dr = {k: nc.inline_tensor(v, name=f"c_{k}") for k, v in cst.items()}

    with tile.TileContext(nc) as tc, ExitStack() as ctx:
        _patch_commit_for_wait_caps(tc, nc)
        # ---- persistent SBUF ----
        cp = ctx.enter_context(tc.tile_pool(name="consts", bufs=1))
        wcat_sb = cp.tile([128, 8 * GC], BF16, tag="wcat")
        wsml_sb = cp.tile([128, 8 * 12], BF16, tag="wsml")
        wout_sb = cp.tile([128, HG * 1024], BF16, tag="wout")
        dtb_sb = cp.tile([128, 4], F32, tag="dtb")
        aneg_sb = cp.tile([128, 4], F32, tag="aneg")
        i2x8_sb = cp.tile([128, 512], F32, tag="i2x8")
        mSL_sb = cp.tile([128, 512], F32, tag="mSL")
        mSU_sb = cp.tile([128, 512], F32, tag="mSU")
        mUI_sb = cp.tile([128, 512], F32, tag="mUI")
        triucum_sb = cp.tile([128, 128], F32, tag="triucum")
        e64sel_sb = cp.tile([128, 128], F32, tag="e64sel")
        identbf_sb = cp.tile([128, 128], BF16, tag="identbf")
        identf_sb = cp.tile([128, 128], F32, tag="identf")
        ones4_sb = cp.tile([128, 4], F32, tag="ones4")
        sh1f_sb = cp.tile([128, 128], F32, tag="sh1f")
        sh1bf_sb = cp.tile([128, 128], BF16, tag="sh1bf")
        sel127f_sb = cp.tile([128, 128], F32, tag="sel127f")
        sel127bf_sb = cp.tile([128, 128], BF16, tag="sel127bf")
        selbf_sb = cp.tile([16, 16 * 128], BF16, tag="selbf")
        sel2f_sb = cp.tile([128, 2 * 128], F32, tag="sel2f")
        S32 = cp.tile([128, HG * HD], F32, tag="S32")
        Sbf = cp.tile([128, HG * HD], BF16, tag="Sbf")

        for nm, t_ in (("i2x8", i2x8_sb), ("mSL", mSL_sb), ("mSU", mSU_sb),
                       ("mUI", mUI_sb), ("triucum", triucum_sb),
                       ("e64sel", e64sel_sb), ("identbf", identbf_sb),
                       ("identf", identf_sb), ("ones4", ones4_sb),
                       ("sh1f", sh1f_sb), ("sh1bf", sh1bf_sb),
                       ("sel127f", sel127f_sb), ("sel127bf", sel127bf_sb),
                       ("selbf", selbf_sb), ("sel2f", sel2f_sb)):
            nc.sync.dma_start(t_[:], dr[nm][:])
        nc.sync.dma_start(wcat_sb[:].rearrange("p (k n) -> p k n", k=8), wcat[:])
        nc.sync.dma_start(wsml_sb[:].rearrange("p (k n) -> p k n", k=8), wsml[:])
        nc.sync.dma_start(wout_sb[:].rearrange("p (h n) -> p h n", h=HG), wout[:])
        nc.sync.dma_start(dtb_sb[:], dtb[:])
        nc.sync.dma_start(aneg_sb[:], aneg[:])
        nc.gpsimd.memset(S32[:], 0.0)
        nc.gpsimd.memset(Sbf[:], 0.0)

        # ---- pools ----
        xT_pool = ctx.enter_context(tc.tile_pool(name="xT", bufs=2))
        xn_pool = ctx.enter_context(tc.tile_pool(name="xn", bufs=2))
        ph1_pool = ctx.enter_context(tc.tile_pool(name="ph1", bufs=1))
        xs_pool = ctx.enter_context(tc.tile_pool(name="xs", bufs=2))
        ph2_pool = ctx.enter_context(tc.tile_pool(name="ph2", bufs=2))
        bun_pool = ctx.enter_context(tc.tile_pool(name="bun", bufs=3))
        tr_pool = ctx.enter_context(tc.tile_pool(name="tr", bufs=2))
        vn_pool = ctx.enter_context(tc.tile_pool(name="vn", bufs=3))
        os_pool = ctx.enter_context(tc.tile_pool(name="os", bufs=2))

        ps_a = ctx.enter_context(tc.tile_pool(name="psA", bufs=2, space="PSUM"))
        ps_b = ctx.enter_context(tc.tile_pool(name="psB", bufs=2, space="PSUM"))
        ps_c = ctx.enter_context(tc.tile_pool(name="psC", bufs=3, space="PSUM"))
        ps_d = ctx.enter_context(tc.tile_pool(name="psD", bufs=2, space="PSUM"))

        def mm(out, lhsT, rhs, start=True, stop=True, tp=None):
            nc.tensor.matmul(out, lhsT, rhs, start=start, stop=stop)

        def mm_q(out, lhsT, rhs, start=True, stop=True):
            # K-operands at partition offset 64 fault at runtime when M=128
            # (full-width row-offset tile); split into two 64-col quadrants.
            if lhsT.base_partition() != 0 and lhsT.free_size() > 64:
                assert lhsT.free_size() == 128
                nc.tensor.matmul(out[0:64, :], lhsT[:, 0:64], rhs,
                                 start=start, stop=stop)
                nc.tensor.matmul(out[64:128, :], lhsT[:, 64:128], rhs,
                                 start=start, stop=stop)
            else:
                nc.tensor.matmul(out, lhsT, rhs, start=start, stop=stop)

        def selbf_mm(out, target, rhs_cols):
            """out[m, t] = rpbf[target, t] broadcast over 128 partitions."""
            mm(out, selbf_sb[:, target * 128:(target + 1) * 128], rhs_cols)

        wcat_v = wcat_sb[:].rearrange("p (k n) -> p k n", k=8)
        wsml_v = wsml_sb[:].rearrange("p (k n) -> p k n", k=8)
        wout_v = wout_sb[:].rearrange("p (h n) -> p h n", h=HG)

        # ---- phase-2 chunk + output projection, deferred one segment ----
        # Segment s's sequential state recurrence is emitted interleaved
        # into segment s+1's token-parallel work so the chunk chain's
        # non-PE latency is covered by phase-1 matmuls.
        def emit_ph2_chunk(cx, n):
            tt, par = n // 2, n % 2
            psl = slice(par * 64, par * 64 + 64)
            pvn = ps_c.tile([128, 256], F32, tag="ph2", name="pvn")
            for h in range(HG):
                qp = slice((h % 2) * 64, (h % 2) * 64 + 64)
                qf = slice((h // 2) * 128, (h // 2) * 128 + 128)
                mm(pvn[qp, qf],
                   cx["ATdv"][psl, h, n * C:(n + 1) * C],
                   cx["vnatv"][psl, tt, h * HD:(h + 1) * HD],
                   start=True, stop=False)
                mm(pvn[qp, qf], cx["wcdTv"][:, h, n * C:(n + 1) * C],
                   Sbf[:, h * HD:(h + 1) * HD],
                   start=False, stop=True)
            vns = vn_pool.tile([128, 256], BF16, tag="vns", name="vns")
            nc.scalar.copy(vns[:], pvn[:])
            vnsD = vn_pool.tile([128, 256], BF16, tag="vnsD", name="vnsD")
            nc.vector.tensor_copy(vnsD[0:64, :], vns[64:128, :])
            nc.vector.tensor_copy(vnsD[64:128, :], vns[0:64, :])
            pot = ps_c.tile([128, 256], F32, tag="ph2", name="pot")
            for h in range(HG):
                qp = slice((h % 2) * 64, (h % 2) * 64 + 64)
                qf = slice((h // 2) * 128, (h // 2) * 128 + 128)
                mm(pot[:, h * 64:(h + 1) * 64],
                   Sbf[:, h * HD:(h + 1) * HD],
                   cx["xTv"][:, h, 1 + n * C:1 + (n + 1) * C],
                   start=True, stop=False)
                mm_q(pot[:, h * 64:(h + 1) * 64], vns[qp, qf],
                     cx["attnTv"][(h % 2) * 64:(h % 2) * 64 + 64, h // 2,
                                  n * C:(n + 1) * C],
                     start=False, stop=True)
            nc.vector.scalar_tensor_tensor(
                cx["oTv"][:, :, n * C:(n + 1) * C],
                cx["gplv"][:, :, n * C:(n + 1) * C], 1.0,
                pot[:].rearrange("p (h t) -> p h t", h=HG),
                op0=OP.mult, op1=OP.mult)
            pS = ps_c.tile([128, 512], F32, tag="ph2", name="pS")
            for h in range(HG):
                qf = slice((h // 2) * 128, (h // 2) * 128 + 128)
                vsrc = vns if (h % 2) == par else vnsD
                mm_q(pS[:, h * HD:(h + 1) * HD],
                     cx["wkdwnv"][psl, tt, h * HD:(h + 1) * HD],
                     vsrc[psl, qf])
            sscr = vn_pool.tile([128, 512], F32, tag="sscr", name="sscr")
            gam_ = cx["gam"]
            gcol = bass.AP(gam_[:].tensor, gam_[:].offset + n,
                           [[HG * NCHS, 128], [NCHS, HG], [0, HD]])
            nc.vector.tensor_tensor(
                sscr[:].rearrange("p (h e) -> p h e", h=HG),
                S32[:].rearrange("p (h e) -> p h e", h=HG),
                gcol, op=OP.mult)
            nc.vector.tensor_add(S32[:], sscr[:], pS[:])
            nc.scalar.copy(Sbf[:], S32[:])

        def emit_outproj(cx):
            t0_ = cx["t0"]
            for dt_ in range(8):
                pop = ps_a.tile([128, 512], F32, tag="vps", name="pop")
                for h in range(HG):
                    mm(pop[:], wout_v[:, h, dt_ * 128:(dt_ + 1) * 128],
                       cx["oTv"][:, h, :], start=(h == 0), stop=(h == 3))
                ob = os_pool.tile([128, 512], BF16, tag="ob", name="ob")
                nc.vector.tensor_copy(ob[:], pop[:])
                nc.sync.dma_start(outp[dt_, :, t0_:t0_ + TSEG], ob[:])

        prev = None
        for s in range(NSEG):
            t0 = s * TSEG
            # ============ loads ============
            xT = xT_pool.tile([128, 8 * (TSEG + 1)], BF16, tag="xT")
            xTv = xT[:].rearrange("p (k t) -> p k t", k=8)
            nc.sync.dma_start(
                xTv[:],
                xth[:, :, t0:t0 + TSEG + 1].rearrange("k p t -> p k t"))
            xn = xn_pool.tile([128, NTILE * GC], BF16, tag="xn")
            xnv = xn[:].rearrange("p (t n) -> p t n", t=NTILE)
            nc.sync.dma_start(
                xnv[:],
                xnh[1 + t0:1 + t0 + TSEG, :].rearrange("(t p) c -> p t c",
                                                       p=128))
            # shifted x (natural): same HBM tensor, one-token-earlier window
            xs = xs_pool.tile([128, NTILE * GC], BF16, tag="xs")
            xsv = xs[:].rearrange("p (t n) -> p t n", t=NTILE)
            nc.sync.dma_start(
                xsv[:],
                xnh[t0:t0 + TSEG, :].rearrange("(t p) c -> p t c", p=128))

            # per-seg tensors
            rpbf = tr_pool.tile([HG * NBF, TSEG], BF16, tag="rpbf")
            rpf32a = tr_pool.tile([128, TSEG], F32, tag="rpf32a")
            rpf32b = tr_pool.tile([128, TSEG], F32, tag="rpf32b")

            def rpf32_rows(h, col0, ncols):
                t_ = rpf32a if h < 2 else rpf32b
                r0 = (h % 2) * 64
                return t_[r0:r0 + 2, col0:col0 + ncols]
            vnat = ph2_pool.tile([128, NTILE * GC], BF16, tag="vnat")
            vnatv = vnat[:].rearrange("p (t n) -> p t n", t=NTILE)
            wkbn = ph2_pool.tile([128, NTILE * GC], BF16, tag="wkbn")
            wkbnv = wkbn[:].rearrange("p (t n) -> p t n", t=NTILE)
            wkdwn = ph2_pool.tile([128, NTILE * GC], BF16, tag="wkdwn")
            wkdwnv = wkdwn[:].rearrange("p (t n) -> p t n", t=NTILE)
            gpl = ph2_pool.tile([128, HG * TSEG], BF16, tag="gpl")
            gplv = gpl[:].rearrange("p (h t) -> p h t", h=HG)
            attnT = ph2_pool.tile([128, (HG // 2) * TSEG], BF16, tag="attnT")
            attnTv = attnT[:].rearrange("p (r n) -> p r n", r=HG // 2)
            ATd = ph2_pool.tile([128, HG * TSEG], BF16, tag="ATd")
            ATdv = ATd[:].rearrange("p (h t) -> p h t", h=HG)
            wcdT = ph2_pool.tile([128, HG * TSEG], BF16, tag="wcdT")
            wcdTv = wcdT[:].rearrange("p (h t) -> p h t", h=HG)
            oT = ph2_pool.tile([128, HG * TSEG], BF16, tag="oT")
            oTv = oT[:].rearrange("p (h t) -> p h t", h=HG)
            gam = tr_pool.tile([128, HG * NCHS], F32, tag="gam")

            # ============ per token-tile: projections + scalar bundle ======
            for tt in range(NTILE):
                psv = ps_a.tile([128, GC], F32, tag="vps")
                pss = ps_d.tile([128, 12], F32, tag="small")
                for kb in range(8):
                    xtt = xTv[:, kb, 1 + tt * 128:1 + (tt + 1) * 128]
                    mm(psv[:], xtt, wcat_v[:, kb, :],
                       start=(kb == 0), stop=(kb == 7))
                for kb in range(8):
                    xtt = xTv[:, kb, 1 + tt * 128:1 + (tt + 1) * 128]
                    mm(pss[:], xtt, wsml_v[:, kb, :],
                       start=(kb == 0), stop=(kb == 7))

                bun = bun_pool.tile([128, HG * NQ], F32, tag="bun")
                bv = bun[:].rearrange("p (h q) -> p h q", h=HG)
                scr = bun_pool.tile([128, 24], F32, tag="scr")
                sq = bun_pool.tile([128, 128], F32, tag="sq")
                # norms -> f (from x) and f_shift (from xs, same pipeline)
                for h in range(HG):
                    nc.scalar.activation(sq[:], xnv[:, tt, h * HD:(h + 1) * HD],
                                         AF.Square, accum_out=scr[:, h:h + 1])
                for h in range(HG):
                    nc.scalar.activation(sq[:], xsv[:, tt, h * HD:(h + 1) * HD],
                                         AF.Square,
                                         accum_out=scr[:, 4 + h:5 + h])
                nc.vector.tensor_scalar_max(scr[:, 8:16], scr[:, 0:8], 1e-24)
                nc.scalar.activation(scr[:, 16:24], scr[:, 8:16], AF.Ln)
                nc.scalar.activation(bv[:, :, QF], scr[:, 16:20], AF.Exp,
                                     scale=-0.5)
                nc.scalar.activation(bv[:, :, QFSH], scr[:, 20:24], AF.Exp,
                                     scale=-0.5)
                # sigmoids
                sg = bun_pool.tile([128, 8], F32, tag="sg")
                nc.scalar.activation(sg[:, 0:4], pss[:, 0:4], AF.Exp,
                                     scale=-1.0)
                nc.scalar.activation(sg[:, 4:8], pss[:, 8:12], AF.Exp,
                                     scale=-1.0)
                nc.vector.tensor_scalar_add(sg[:, 0:8], sg[:, 0:8], 1.0)
                nc.vector.reciprocal(bv[:, :, QBETA], sg[:, 0:4])
                nc.vector.reciprocal(bv[:, :, QG], sg[:, 4:8])
                # decay
                nc.vector.tensor_add(scr[:, 12:16], pss[:, 4:8], dtb_sb[:])
                nc.scalar.activation(scr[:, 16:20], scr[:, 12:16], AF.Exp)
                nc.scalar.activation(scr[:, 16:20], scr[:, 16:20], AF.Ln,
                                     bias=1.0)
                nc.vector.tensor_mul(scr[:, 20:24], scr[:, 16:20], aneg_sb[:])
                # within-chunk cumulative decay
                psc = ps_d.tile([128, 4], F32, tag="small")
                mm(psc[:], triucum_sb[:], scr[:, 20:24])
                nc.scalar.copy(bv[:, :, QDEC], psc[:])
                psl = ps_d.tile([128, 4], F32, tag="small")
                mm(psl[:], e64sel_sb[:], bv[:, :, QDEC])
                nc.vector.tensor_sub(scr[:, 0:4], psl[:], bv[:, :, QDEC])
                nc.scalar.activation(scr[:, 4:8], scr[:, 0:4], AF.Exp)  # dw
                nc.scalar.activation(bv[:, :, QEDEC], bv[:, :, QDEC], AF.Exp)
                nc.scalar.activation(bv[:, :, QEDEC2], bv[:, :, QDEC], AF.Exp)
                nc.scalar.activation(bv[:, :, QEDECI], bv[:, :, QDEC], AF.Exp,
                                     scale=-1.0)
                nc.vector.tensor_mul(bv[:, :, QRA], bv[:, :, QF],
                                     bv[:, :, QEDEC])
                # fold f*edec into the gate: the rk-side per-token factor is
                # applied to pot's output columns via gpl instead of to xT
                nc.vector.tensor_mul(bv[:, :, QG], bv[:, :, QG],
                                     bv[:, :, QRA])
                nc.vector.tensor_mul(scr[:, 8:12], bv[:, :, QFSH],
                                     bv[:, :, QBETA])
                nc.vector.tensor_mul(bv[:, :, QFB], scr[:, 8:12],
                                     bv[:, :, QEDEC])
                nc.vector.tensor_mul(bv[:, :, QWB], bv[:, :, QFSH],
                                     bv[:, :, QEDECI])
                nc.vector.tensor_mul(bv[:, :, QFDW], bv[:, :, QFSH],
                                     scr[:, 4:8])
                # transposed per-token scalars
                bsh = bun_pool.tile([128, HG * NBF], BF16, tag="bsh")
                nc.vector.tensor_copy(
                    bsh[:].rearrange("p (h q) -> p h q", h=HG),
                    bv[:, :, 0:NBF])
                ptb = ps_d.tile([HG * NBF, 128], BF16, tag="small")
                nc.tensor.transpose(ptb[:], bsh[:], identbf_sb[:])
                nc.scalar.copy(rpbf[:, tt * 128:(tt + 1) * 128], ptb[:])
                for h in range(HG):
                    ptf = ps_d.tile([2, 128], F32, tag="small")
                    nc.tensor.transpose(
                        ptf[:], bun[:, h * NQ + QDEC:h * NQ + QDEC + 2],
                        identf_sb[:])
                    nc.scalar.copy(rpf32_rows(h, tt * 128, 128), ptf[:])
                # v' = beta * v  (natural, bf16)
                nc.vector.scalar_tensor_tensor(
                    vnatv[:, tt, :].rearrange("p (h e) -> p h e", h=HG),
                    psv[:].rearrange("p (h e) -> p h e", h=HG), 1.0,
                    bv[:, :, QBETA].broadcast_to((128, HG, HD)),
                    op0=OP.mult, op1=OP.mult)
                # shifted-key natural tensors
                nc.vector.scalar_tensor_tensor(
                    wkbnv[:, tt, :].rearrange("p (h e) -> p h e", h=HG),
                    xsv[:, tt, :].rearrange("p (h e) -> p h e", h=HG), 1.0,
                    bv[:, :, QFB].broadcast_to((128, HG, HD)),
                    op0=OP.mult, op1=OP.mult)
                nc.vector.scalar_tensor_tensor(
                    wkdwnv[:, tt, :].rearrange("p (h e) -> p h e", h=HG),
                    xsv[:, tt, :].rearrange("p (h e) -> p h e", h=HG), 1.0,
                    bv[:, :, QFDW].broadcast_to((128, HG, HD)),
                    op0=OP.mult, op1=OP.mult)

            if stage < 2:
                continue
            # ============ per head: T-side tiles + gamma ============
            wkA_l, wkB_l = [], []
            for h in range(HG):
                wkA = ph1_pool.tile([128, TSEG], BF16, tag=f"wkA{h}")
                wkB = ph1_pool.tile([128, TSEG], BF16, tag=f"wkB{h}")
                wkA_l.append(wkA); wkB_l.append(wkB)

                rp16 = rpbf[0:16, :]
                pa = ps_b.tile([128, TSEG], F32, tag="prod")
                selbf_mm(pa[:], h * NBF + QFB, rp16)
                nc.vector.scalar_tensor_tensor(
                    wkA[:], xTv[:, h, 0:TSEG], 1.0, pa[:],
                    op0=OP.mult, op1=OP.mult)
                pb = ps_b.tile([128, TSEG], F32, tag="prod")
                selbf_mm(pb[:], h * NBF + QWB, rp16)
                nc.vector.scalar_tensor_tensor(
                    wkB[:], xTv[:, h, 0:TSEG], 1.0, pb[:],
                    op0=OP.mult, op1=OP.mult)
                pg = ps_b.tile([128, TSEG], F32, tag="prod")
                selbf_mm(pg[:], h * NBF + QG, rp16)
                nc.scalar.copy(gplv[:, h, :], pg[:])
                # gamma = exp(dec at chunk end), broadcast to all partitions
                pgm = ps_d.tile([128, NCHS], F32, tag="small")
                rt = rpf32a if h < 2 else rpf32b
                r0 = (h % 2) * 64
                gsel2 = bass.AP(rt[:].tensor,
                                rt[:].offset + (h % 2) * 64 * TSEG + 63,
                                [[TSEG, 2], [C, NCHS]])
                mm(pgm[:], sel2f_sb[r0:r0 + 2, 128:256], gsel2)
                nc.scalar.copy(gam[:, h * NCHS:(h + 1) * NCHS], pgm[:])

            if stage < 3:
                continue
            # ===== phase-1: products, masks, 3-factor inverse =====
            # A.T = ((I-M)(I+M^2)(I+M^4)).T, error O(M^8); the two head
            # pairs (pr) are interleaved stage-by-stage so PE never waits
            # on a PSUM drain (the other pair's matmuls cover it).
            def chunk_mms(out_ps, lh, rh, pr):
                for hh in range(2):
                    sl = slice(hh * 64, (hh + 1) * 64)
                    for n in range(NCHS):
                        csl = slice(n * C, (n + 1) * C)
                        mm(out_ps[sl, csl], lh[sl, csl], rh[sl, csl])

            Msb_, MTsb_, ImM_, P1r_, P1i_, Q1r_, G0_, G1_ = ({} for _ in
                                                             range(8))
            for pr in range(2):
                pp1 = ps_b.tile([128, 512], F32, tag="prod")
                for hh in range(2):
                    h = pr * 2 + hh
                    sl = slice(hh * 64, (hh + 1) * 64)
                    for n in range(NCHS):
                        csl = slice(n * C, (n + 1) * C)
                        mm(pp1[sl, csl], wkA_l[h][:, csl], wkB_l[h][:, csl])
                Msb_[pr] = ph1_pool.tile([128, 512], BF16, tag=f"Msb{pr}",
                               name=f"Msb{pr}")
                nc.vector.scalar_tensor_tensor(Msb_[pr][:], pp1[:], 1.0,
                                               mSL_sb[:],
                                               op0=OP.mult, op1=OP.mult)
            for pr in range(2):
                pp1t = ps_b.tile([128, 512], F32, tag="prod")
                for hh in range(2):
                    h = pr * 2 + hh
                    sl = slice(hh * 64, (hh + 1) * 64)
                    for n in range(NCHS):
                        csl = slice(n * C, (n + 1) * C)
                        mm(pp1t[sl, csl], wkB_l[h][:, csl], wkA_l[h][:, csl])
                MTsb_[pr] = ph1_pool.tile([128, 512], BF16, tag=f"MTsb{pr}",
                                name=f"MTsb{pr}")
                nc.vector.scalar_tensor_tensor(MTsb_[pr][:], pp1t[:], 1.0,
                                               mSU_sb[:],
                                               op0=OP.mult, op1=OP.mult)
                ImM_[pr] = ph1_pool.tile([128, 512], BF16, tag=f"ImM{pr}",
                               name=f"ImM{pr}")
                nc.vector.scalar_tensor_tensor(ImM_[pr][:], Msb_[pr][:], -1.0,
                                               i2x8_sb[:],
                                               op0=OP.mult, op1=OP.add)
            for pr in range(2):
                pp2 = ps_b.tile([128, 512], F32, tag="prod")
                for hh in range(2):
                    h = pr * 2 + hh
                    sl = slice(hh * 64, (hh + 1) * 64)
                    for n in range(NCHS):
                        csl = slice(n * C, (n + 1) * C)
                        mm(pp2[sl, csl], wkB_l[h][:, csl],
                           xTv[:, h, 1 + n * C:1 + (n + 1) * C])
                nc.vector.scalar_tensor_tensor(attnTv[:, pr, :], pp2[:], 1.0,
                                               mUI_sb[:],
                                               op0=OP.mult, op1=OP.mult)
            for pr in range(2):
                pP1 = ps_b.tile([128, 512], F32, tag="prod")
                chunk_mms(pP1, MTsb_[pr][:], Msb_[pr][:], pr)
                P1r_[pr] = ph1_pool.tile([128, 512], BF16, tag=f"P1r{pr}",
                               name=f"P1r{pr}")
                P1i_[pr] = ph1_pool.tile([128, 512], BF16, tag=f"P1i{pr}",
                               name=f"P1i{pr}")
                nc.scalar.copy(P1r_[pr][:], pP1[:])
                nc.vector.scalar_tensor_tensor(P1i_[pr][:], pP1[:], 1.0,
                                               i2x8_sb[:],
                                               op0=OP.mult, op1=OP.add)
            for pr in range(2):
                pQ1 = ps_b.tile([128, 512], F32, tag="prod")
                chunk_mms(pQ1, Msb_[pr][:], MTsb_[pr][:], pr)
                Q1r_[pr] = ph1_pool.tile([128, 512], BF16, tag=f"Q1r{pr}",
                               name=f"Q1r{pr}")
                nc.scalar.copy(Q1r_[pr][:], pQ1[:])
            P2r_, P2i_, Q2r_, G2_ = {}, {}, {}, {}
            for pr in range(2):
                pP2 = ps_b.tile([128, 512], F32, tag="prod")
                chunk_mms(pP2, Q1r_[pr][:], P1r_[pr][:], pr)
                P2r_[pr] = ph1_pool.tile([128, 512], BF16, tag=f"P2r{pr}",
                                         name=f"P2r{pr}")
                P2i_[pr] = ph1_pool.tile([128, 512], BF16, tag=f"P2i{pr}",
                                         name=f"P2i{pr}")
                nc.scalar.copy(P2r_[pr][:], pP2[:])
                nc.vector.scalar_tensor_tensor(P2i_[pr][:], pP2[:], 1.0,
                                               i2x8_sb[:],
                                               op0=OP.mult, op1=OP.add)
            for pr in range(2):
                pQ2 = ps_b.tile([128, 512], F32, tag="prod")
                chunk_mms(pQ2, P1r_[pr][:], Q1r_[pr][:], pr)
                Q2r_[pr] = ph1_pool.tile([128, 512], BF16, tag=f"Q2r{pr}",
                                         name=f"Q2r{pr}")
                nc.scalar.copy(Q2r_[pr][:], pQ2[:])
            for pr in range(2):
                pQ3 = ps_b.tile([128, 512], F32, tag="prod")
                chunk_mms(pQ3, P2r_[pr][:], Q2r_[pr][:], pr)
                G0_[pr] = ph1_pool.tile([128, 512], BF16, tag=f"G0{pr}",
                              name=f"G0{pr}")
                nc.vector.scalar_tensor_tensor(G0_[pr][:], pQ3[:], 1.0,
                                               i2x8_sb[:],
                                               op0=OP.mult, op1=OP.add)
            for pr in range(2):
                pG1 = ps_b.tile([128, 512], F32, tag="prod")
                chunk_mms(pG1, P2i_[pr][:], G0_[pr][:], pr)
                G1_[pr] = ph1_pool.tile([128, 512], BF16, tag=f"G1{pr}",
                              name=f"G1{pr}")
                nc.scalar.copy(G1_[pr][:], pG1[:])
            for pr in range(2):
                pG2 = ps_b.tile([128, 512], F32, tag="prod")
                chunk_mms(pG2, P1i_[pr][:], G1_[pr][:], pr)
                G2_[pr] = ph1_pool.tile([128, 512], BF16, tag=f"G2{pr}",
                                        name=f"G2{pr}")
                nc.scalar.copy(G2_[pr][:], pG2[:])
            for pr in range(2):
                pAT = ps_b.tile([128, 512], F32, tag="prod")
                chunk_mms(pAT, ImM_[pr][:], G2_[pr][:], pr)
                # duplicate each chunk's AT at both partition parities
                for hh in range(2):
                    h = pr * 2 + hh
                    for par in range(2):
                        nc.scalar.copy(
                            ATdv[par * 64:(par + 1) * 64, h, :].rearrange(
                                "p (n c) -> p n c", c=C)[:, par::2, :],
                            pAT[hh * 64:(hh + 1) * 64, :].rearrange(
                                "p (n c) -> p n c", c=C)[:, par::2, :])

            if stage < 4:
                continue
            # wk_cumdecay.T = -(A @ wkb')^T per (head, chunk)
            for h in range(HG):
                pwc = ps_b.tile([128, 512], F32, tag="prod")
                for n in range(NCHS):
                    mm_q(pwc[:, n * C:(n + 1) * C],
                       wkbnv[(n % 2) * 64:(n % 2) * 64 + 64, n // 2,
                             h * HD:(h + 1) * HD],
                       ATdv[(n % 2) * 64:(n % 2) * 64 + 64, h,
                            n * C:(n + 1) * C])
                nc.vector.tensor_scalar_mul(wcdTv[:, h, :], pwc[:], -1.0)

            if stage < 4.5:
                continue
            # ============ phase 2: sequential chunk recurrence ============
            for n in range(NCHS):
                tt, par = n // 2, n % 2
                psl = slice(par * 64, par * 64 + 64)
                pvn = ps_c.tile([128, 256], F32, tag="ph2")
                for h in range(HG):
                    qp = slice((h % 2) * 64, (h % 2) * 64 + 64)
                    qf = slice((h // 2) * 128, (h // 2) * 128 + 128)
                    mm(pvn[qp, qf],
                       ATdv[psl, h, n * C:(n + 1) * C],
                       vnatv[psl, tt, h * HD:(h + 1) * HD],
                       start=True, stop=False)
                    mm(pvn[qp, qf], wcdTv[:, h, n * C:(n + 1) * C],
                       Sbf[:, h * HD:(h + 1) * HD],
                       start=False, stop=True)
                vns = vn_pool.tile([128, 256], BF16, tag="vns")
                nc.scalar.copy(vns[:], pvn[:])
                vnsD = vn_pool.tile([128, 256], BF16, tag="vnsD")
                nc.vector.tensor_copy(vnsD[0:64, :], vns[64:128, :])
                nc.vector.tensor_copy(vnsD[64:128, :], vns[0:64, :])
                pot = ps_c.tile([128, 256], F32, tag="ph2")
                for h in range(HG):
                    qp = slice((h % 2) * 64, (h % 2) * 64 + 64)
                    qf = slice((h // 2) * 128, (h // 2) * 128 + 128)
                    mm(pot[:, h * 64:(h + 1) * 64],
                       Sbf[:, h * HD:(h + 1) * HD],
                       xTv[:, h, 1 + n * C:1 + (n + 1) * C],
                       start=True, stop=False)
                    mm_q(pot[:, h * 64:(h + 1) * 64], vns[qp, qf],
                         attnTv[(h % 2) * 64:(h % 2) * 64 + 64, h // 2,
                                n * C:(n + 1) * C],
                         start=False, stop=True)
                nc.vector.scalar_tensor_tensor(
                    oTv[:, :, n * C:(n + 1) * C],
                    gplv[:, :, n * C:(n + 1) * C], 1.0,
                    pot[:].rearrange("p (h t) -> p h t", h=HG),
                    op0=OP.mult, op1=OP.mult)
                pS = ps_c.tile([128, 512], F32, tag="ph2")
                for h in range(HG):
                    qf = slice((h // 2) * 128, (h // 2) * 128 + 128)
                    vsrc = vns if (h % 2) == par else vnsD
                    mm_q(pS[:, h * HD:(h + 1) * HD],
                         wkdwnv[psl, tt, h * HD:(h + 1) * HD], vsrc[psl, qf])
                sscr = vn_pool.tile([128, 512], F32, tag="sscr")
                gcol = bass.AP(gam[:].tensor, gam[:].offset + n,
                               [[HG * NCHS, 128], [NCHS, HG], [0, HD]])
                nc.vector.tensor_tensor(
                    sscr[:].rearrange("p (h e) -> p h e", h=HG),
                    S32[:].rearrange("p (h e) -> p h e", h=HG),
                    gcol, op=OP.mult)
                nc.vector.tensor_add(S32[:], sscr[:], pS[:])
                nc.scalar.copy(Sbf[:], S32[:])

            if stage < 5:
                continue
            # ============ output projection ============
            for dt_ in range(8):
                pop = ps_a.tile([128, 512], F32, tag="vps")
                for h in range(HG):
                    mm(pop[:], wout_v[:, h, dt_ * 128:(dt_ + 1) * 128],
                       oTv[:, h, :], start=(h == 0), stop=(h == 3))
                ob = os_pool.tile([128, 512], F32, tag="ob")
                nc.vector.tensor_copy(ob[:], pop[:])
                nc.sync.dma_start(outp[dt_, :, t0:t0 + TSEG], ob[:])

    return nc


def _merge_waits(waits):
    """Merge duplicate-sem waits keeping the max threshold (sem-ge modes)."""
    best, order = {}, []
    for w in waits:
        k = getattr(w, "ant_name", None) or str(getattr(w, "id", ""))
        if k not in best:
            best[k] = w
            order.append(k)
        elif (getattr(w, "wait_value", 0) or 0) > (getattr(best[k], "wait_value", 0) or 0):
            best[k] = w
    return [best[k] for k in order]


def _patch_commit_for_wait_caps(tc, nc, cap=1):
    """Wrap TileContext._commit_instruction: instructions whose wait list
    exceeds the ISA sync-slot budget get standalone EventSemaphore carriers
    emitted immediately before them on the same engine."""
    orig = tc._commit_instruction

    def patched(inst, lazy_reg_writes=True):
        si = getattr(inst, "sync_info", None)
        eng = getattr(inst, "engine", None)
        if si is not None and si.on_wait and eng is not None:
            w = _merge_waits(list(si.on_wait))
            if len(w) > cap:
                keep, excess = w[:cap], w[cap:]
                for ww in excess:
                    ev = mybir.InstDrain(
                        name=nc.get_next_instruction_name(),
                        ins=[], outs=[],
                        sync_info=mybir.SyncInfo(on_wait=[ww], on_update=[]))
                    ev.engine = eng
                    orig(ev, lazy_reg_writes=False)
                w = keep
            if len(w) != len(si.on_wait):
                inst.sync_info = mybir.SyncInfo(
                    on_wait=w, on_update=list(si.on_update or []))
        return orig(inst, lazy_reg_writes)

    tc._commit_instruction = patched

    orig_dab = tc._drain_and_barrier

    def patched_dab(tick_clock, wait_clock):
        from concourse.tile import ScopedClock
        d = nc.sync.drain()
        wait_clock.add_sem_waits(
            d.ins, ScopedClock({None: tick_clock.global_clock}))
        si = d.ins.sync_info
        if si is not None and si.on_wait and len(si.on_wait) > 1:
            extra = list(si.on_wait[1:])
            d.ins.sync_info = mybir.SyncInfo(
                on_wait=[si.on_wait[0]],
                on_update=list(si.on_update or []))
            for w in extra:
                d2 = nc.sync.drain()
                d2.ins.sync_info = mybir.SyncInfo(on_wait=[w], on_update=[])
        nc.all_engine_barrier()
        popped = nc._tile_sem_poison_stack.pop()
        assert popped is tc._sem_poison
        nc.clear_and_free_semaphores(list(tc.sems.allocated().values()))
        nc.all_engine_barrier()

    tc._drain_and_barrier = patched_dab


# ---------------- host side ----------------

def _prep_core_inputs(x_b, g, W_write, W_gate, W_out, W_beta, W_alpha,
                      dt_bias, A_log, Ttot):
    perm = np.arange(D) if g == 0 else np.concatenate(
        [np.arange(GC, 2 * GC), np.arange(0, GC)])
    xr = x_b[:, perm]
    hsl = slice(g * HG, (g + 1) * HG)
    Ww = W_write[g * GC:(g + 1) * GC, :][:, perm]
    Wsml = np.concatenate([W_beta[hsl], W_alpha[hsl], W_gate[hsl]], 0)[:, perm]
    Wo = W_out[:, g * GC:(g + 1) * GC]

    wcat_np = np.ascontiguousarray(
        Ww.T.reshape(8, 128, GC).transpose(1, 0, 2)).astype(ml_dtypes.bfloat16)
    wsml_np = np.ascontiguousarray(
        Wsml.T.reshape(8, 128, 12).transpose(1, 0, 2)).astype(ml_dtypes.bfloat16)
    wout_np = np.ascontiguousarray(
        Wo.T.reshape(HG, 128, 1024).transpose(1, 0, 2)).astype(ml_dtypes.bfloat16)
    dtb_np = np.broadcast_to(dt_bias[hsl], (128, HG)).astype(np.float32)
    aneg_np = np.broadcast_to(-np.exp(A_log[hsl]), (128, HG)).astype(np.float32)
    xb = xr[:Ttot].astype(ml_dtypes.bfloat16)
    xthn = np.zeros((8, 128, Ttot + 1), ml_dtypes.bfloat16)
    xthn[:, :, 1:] = np.ascontiguousarray(xb.T).reshape(8, 128, Ttot)
    xnhp = np.zeros((Ttot + 1, GC), ml_dtypes.bfloat16)
    xnhp[1:] = xb[:, 0:GC]
    return {
        "xth": xthn,
        "xnh": xnhp,
        "wcat": wcat_np, "wsml": wsml_np, "wout": wout_np,
        "dtb": np.ascontiguousarray(dtb_np),
        "aneg": np.ascontiguousarray(aneg_np),
    }


_NC_CACHE = {}


def kernel(x, W_write, W_gate, W_out, W_beta, W_alpha, dt_bias, A_log,
           _trace=False):
    from concourse.bass_utils import run_bass_kernel_spmd

    x = np.asarray(x)
    Bn, Tn, Dm = x.shape
    if Tn not in _NC_CACHE:
        _NC_CACHE[Tn] = build_nc(Ttot=Tn)
    nc = _NC_CACHE[Tn]

    in_maps = []
    for core in range(NCORES):
        b, g = core // 2, core % 2
        in_maps.append(_prep_core_inputs(
            np.asarray(x[b]), g, np.asarray(W_write), np.asarray(W_gate),
            np.asarray(W_out), np.asarray(W_beta), np.asarray(W_alpha),
            np.asarray(dt_bias), np.asarray(A_log), Tn))

    res = run_bass_kernel_spmd(nc, in_maps, core_ids=list(range(NCORES)),
                               trace=_trace)
    out = np.empty((Bn, Tn, Dm), np.float32)
    for b in range(Bn):
        p0 = res.results[2 * b]["outp"].reshape(Dm, Tn)
        p1 = res.results[2 * b + 1]["outp"].reshape(Dm, Tn)
        out[b] = x[b] + p0.T + p1.T
    if _trace:
        kernel._last_results = res
    return out



# revision 24
# speedup vs baseline: 1.1644x; 1.1644x over previous
"""Trainium2 Bass kernel for the DeltaHebbian (gated delta-rule) block.

Sharding: 8 cores = 4 batches x 2 head-groups (4 heads each). Each core gets
its batch's x with columns rotated so its head-group occupies cols 0:512, and
computes partial_out.T = (gated_o @ W_out_slice.T).T.  Host sums the two
partials per batch and adds x.

Per-core algorithm (chunked delta rule, CHUNK=64):
  phase 1 (token-parallel): projections, key normalization, per-chunk decay
  cumsums, masked key-product matrices M / M.T / attn.T, and the UT-transform
  inverse A.T = ((I+M)^-1).T via the telescoping factorization
  (I-M)(I+M^2)(I+M^4)(I+M^8)  (exact to ~4e-5 on this data: |M^16| ~ 5e-5).
  phase 2 (sequential over chunks, 4 heads interleaved): the state recurrence.
"""

import sys

for _p in ("/opt/trn_rl_repo",):
    if _p not in sys.path:
        sys.path.append(_p)

from contextlib import ExitStack

import numpy as np
import ml_dtypes

import concourse.bass as bass
import concourse.mybir as mybir
import concourse.tile as tile

F32 = mybir.dt.float32
BF16 = mybir.dt.bfloat16
OP = mybir.AluOpType
AF = mybir.ActivationFunctionType

# problem constants
B, T, D = 4, 8192, 1024
HD = 128          # head dim
C = 64            # chunk length
HG = 4            # heads per core
GC = HG * HD      # 512 group columns
NCORES = 8
NQ = 12           # bundle quantities per head
# bundle column indices (per head, stride NQ); cols 0..3 are the bf16
# plane factors (transposed then token-broadcast)
(QRA, QFB, QWB, QG, QF, QFSH, QEDEC, QEDECI, QDEC, QEDEC2, QBETA,
 QFDW) = range(12)
NBF = 4           # bf16 transposed rows per head: cols 0..3


def _consts():
    ii = np.arange(128)
    jj = np.arange(512)
    pi = ii[:, None] % 64
    qi = jj[None, :] % 64
    c = {}
    c["i2x8"] = (pi == qi).astype(np.float32)
    c["mSL"] = (pi > qi).astype(np.float32)      # keep i>j   (M)
    c["mSU"] = (qi > pi).astype(np.float32)      # keep j>i   (M.T)
    c["mUI"] = (qi >= pi).astype(np.float32)     # keep i>=j  (attn.T)
    k = np.arange(128)
    m = np.arange(128)
    same = (k[:, None] // 64) == (m[None, :] // 64)
    c["triucum"] = (same & ((k[:, None] % 64) <= (m[None, :] % 64))).astype(np.float32)
    c["e64sel"] = (k[:, None] == (m[None, :] // 64) * 64 + 63).astype(np.float32)
    c["identbf"] = np.eye(128).astype(ml_dtypes.bfloat16)
    c["identf"] = np.eye(128).astype(np.float32)
    c["ones4"] = np.ones((128, 4), np.float32)
    sh1 = (k[:, None] == m[None, :] - 1).astype(np.float32)   # out[m]=in[m-1]
    c["sh1f"] = sh1
    c["sh1bf"] = sh1.astype(ml_dtypes.bfloat16)
    s127 = np.zeros((128, 128), np.float32)   # out row0 = in row127, rest += 0
    s127[127, 0] = 1.0
    c["sel127f"] = s127
    c["sel127bf"] = s127.astype(ml_dtypes.bfloat16)
    # bf16 row selectors: target t -> (16, 128) block with row t all-ones
    selbf = np.zeros((16, 16 * 128), np.float32)
    for t in range(16):
        selbf[t, t * 128:(t + 1) * 128] = 1.0
    c["selbf"] = selbf.astype(ml_dtypes.bfloat16)
    sel2 = np.zeros((128, 2 * 128), np.float32)  # [dec-sel | edec-sel] at rows 0/64
    for hh in (0, 64):
        sel2[hh + 0, 0:128] = 1.0
        sel2[hh + 1, 128:256] = 1.0
    c["sel2f"] = sel2
    return c


def build_nc(Ttot=T, TSEG=512, stage=5):
    assert Ttot % TSEG == 0 and TSEG == 512
    NSEG = Ttot // TSEG
    NTILE = TSEG // 128
    NCHS = TSEG // C

    nc = bass.Bass()
    xth = nc.dram_tensor("xth", (8, 128, Ttot + 1), BF16, kind="ExternalInput")
    xnh = nc.dram_tensor("xnh", (Ttot + 1, GC), BF16, kind="ExternalInput")
    wcat = nc.dram_tensor("wcat", (128, 8, GC), BF16, kind="ExternalInput")
    wsml = nc.dram_tensor("wsml", (128, 8, 12), BF16, kind="ExternalInput")
    wout = nc.dram_tensor("wout", (128, HG, 1024), BF16, kind="ExternalInput")
    dtb = nc.dram_tensor("dtb", (128, 4), F32, kind="ExternalInput")
    aneg = nc.dram_tensor("aneg", (128, 4), F32, kind="ExternalInput")
    outp = nc.dram_tensor("outp", (8, 128, Ttot), BF16, kind="ExternalOutput")

    cst = _consts()
    dr = {k: nc.inline_tensor(v, name=f"c_{k}") for k, v in cst.items()}

    with tile.TileContext(nc) as tc, ExitStack() as ctx:
        _patch_commit_for_wait_caps(tc, nc)
        # ---- persistent SBUF ----
        cp = ctx.enter_context(tc.tile_pool(name="consts", bufs=1))
        wcat_sb = cp.tile([128, 8 * GC], BF16, tag="wcat")
        wsml_sb = cp.tile([128, 8 * 12], BF16, tag="wsml")
        wout_sb = cp.tile([128, HG * 1024], BF16, tag="wout")
        dtb_sb = cp.tile([128, 4], F32, tag="dtb")
        aneg_sb = cp.tile([128, 4], F32, tag="aneg")
        i2x8_sb = cp.tile([128, 512], F32, tag="i2x8")
        mSL_sb = cp.tile([128, 512], F32, tag="mSL")
        mSU_sb = cp.tile([128, 512], F32, tag="mSU")
        mUI_sb = cp.tile([128, 512], F32, tag="mUI")
        triucum_sb = cp.tile([128, 128], F32, tag="triucum")
        e64sel_sb = cp.tile([128, 128], F32, tag="e64sel")
        identbf_sb = cp.tile([128, 128], BF16, tag="identbf")
        identf_sb = cp.tile([128, 128], F32, tag="identf")
        ones4_sb = cp.tile([128, 4], F32, tag="ones4")
        sh1f_sb = cp.tile([128, 128], F32, tag="sh1f")
        sh1bf_sb = cp.tile([128, 128], BF16, tag="sh1bf")
        sel127f_sb = cp.tile([128, 128], F32, tag="sel127f")
        sel127bf_sb = cp.tile([128, 128], BF16, tag="sel127bf")
        selbf_sb = cp.tile([16, 16 * 128], BF16, tag="selbf")
        sel2f_sb = cp.tile([128, 2 * 128], F32, tag="sel2f")
        S32 = cp.tile([128, HG * HD], F32, tag="S32")
        Sbf = cp.tile([128, HG * HD], BF16, tag="Sbf")

        for nm, t_ in (("i2x8", i2x8_sb), ("mSL", mSL_sb), ("mSU", mSU_sb),
                       ("mUI", mUI_sb), ("triucum", triucum_sb),
                       ("e64sel", e64sel_sb), ("identbf", identbf_sb),
                       ("identf", identf_sb), ("ones4", ones4_sb),
                       ("sh1f", sh1f_sb), ("sh1bf", sh1bf_sb),
                       ("sel127f", sel127f_sb), ("sel127bf", sel127bf_sb),
                       ("selbf", selbf_sb), ("sel2f", sel2f_sb)):
            nc.sync.dma_start(t_[:], dr[nm][:])
        nc.sync.dma_start(wcat_sb[:].rearrange("p (k n) -> p k n", k=8), wcat[:])
        nc.sync.dma_start(wsml_sb[:].rearrange("p (k n) -> p k n", k=8), wsml[:])
        nc.sync.dma_start(wout_sb[:].rearrange("p (h n) -> p h n", h=HG), wout[:])
        nc.sync.dma_start(dtb_sb[:], dtb[:])
        nc.sync.dma_start(aneg_sb[:], aneg[:])
        nc.gpsimd.memset(S32[:], 0.0)
        nc.gpsimd.memset(Sbf[:], 0.0)

        # ---- pools ----
        xT_pool = ctx.enter_context(tc.tile_pool(name="xT", bufs=2))
        xn_pool = ctx.enter_context(tc.tile_pool(name="xn", bufs=2))
        ph1_pool = ctx.enter_context(tc.tile_pool(name="ph1", bufs=1))
        xs_pool = ctx.enter_context(tc.tile_pool(name="xs", bufs=2))
        ph2_pool = ctx.enter_context(tc.tile_pool(name="ph2", bufs=2))
        bun_pool = ctx.enter_context(tc.tile_pool(name="bun", bufs=3))
        tr_pool = ctx.enter_context(tc.tile_pool(name="tr", bufs=2))
        vn_pool = ctx.enter_context(tc.tile_pool(name="vn", bufs=3))
        os_pool = ctx.enter_context(tc.tile_pool(name="os", bufs=2))

        ps_a = ctx.enter_context(tc.tile_pool(name="psA", bufs=2, space="PSUM"))
        ps_b = ctx.enter_context(tc.tile_pool(name="psB", bufs=2, space="PSUM"))
        ps_c = ctx.enter_context(tc.tile_pool(name="psC", bufs=3, space="PSUM"))
        ps_d = ctx.enter_context(tc.tile_pool(name="psD", bufs=1, space="PSUM"))

        def mm(out, lhsT, rhs, start=True, stop=True, tp=None):
            nc.tensor.matmul(out, lhsT, rhs, start=start, stop=stop)

        def mm_q(out, lhsT, rhs, start=True, stop=True):
            # K-operands at partition offset 64 fault at runtime when M=128
            # (full-width row-offset tile); split into two 64-col quadrants.
            if lhsT.base_partition() != 0 and lhsT.free_size() > 64:
                assert lhsT.free_size() == 128
                nc.tensor.matmul(out[0:64, :], lhsT[:, 0:64], rhs,
                                 start=start, stop=stop)
                nc.tensor.matmul(out[64:128, :], lhsT[:, 64:128], rhs,
                                 start=start, stop=stop)
            else:
                nc.tensor.matmul(out, lhsT, rhs, start=start, stop=stop)

        def selbf_mm(out, target, rhs_cols):
            """out[m, t] = rpbf[target, t] broadcast over 128 partitions."""
            mm(out, selbf_sb[:, target * 128:(target + 1) * 128], rhs_cols)

        wcat_v = wcat_sb[:].rearrange("p (k n) -> p k n", k=8)
        wsml_v = wsml_sb[:].rearrange("p (k n) -> p k n", k=8)
        wout_v = wout_sb[:].rearrange("p (h n) -> p h n", h=HG)

        # ---- phase-2 chunk + output projection, deferred one segment ----
        # Segment s's sequential state recurrence is emitted interleaved
        # into segment s+1's token-parallel work so the chunk chain's
        # non-PE latency is covered by phase-1 matmuls.
        def emit_ph2_chunk(cx, n):
            tt, par = n // 2, n % 2
            psl = slice(par * 64, par * 64 + 64)
            pvn = ps_c.tile([128, 256], F32, tag="ph2", name="pvn")
            for h in range(HG):
                qp = slice((h % 2) * 64, (h % 2) * 64 + 64)
                qf = slice((h // 2) * 128, (h // 2) * 128 + 128)
                mm(pvn[qp, qf],
                   cx["ATdv"][psl, h, n * C:(n + 1) * C],
                   cx["vnatv"][psl, tt, h * HD:(h + 1) * HD],
                   start=True, stop=False)
                mm(pvn[qp, qf], cx["wcdTv"][:, h, n * C:(n + 1) * C],
                   Sbf[:, h * HD:(h + 1) * HD],
                   start=False, stop=True)
            vns = vn_pool.tile([128, 256], BF16, tag="vns", name="vns")
            nc.scalar.copy(vns[:], pvn[:])
            vnsD = vn_pool.tile([128, 256], BF16, tag="vnsD", name="vnsD")
            nc.vector.tensor_copy(vnsD[0:64, :], vns[64:128, :])
            nc.vector.tensor_copy(vnsD[64:128, :], vns[0:64, :])
            pot = ps_c.tile([128, 256], F32, tag="ph2", name="pot")
            for h in range(HG):
                qp = slice((h % 2) * 64, (h % 2) * 64 + 64)
                qf = slice((h // 2) * 128, (h // 2) * 128 + 128)
                mm(pot[:, h * 64:(h + 1) * 64],
                   Sbf[:, h * HD:(h + 1) * HD],
                   cx["xTv"][:, h, 1 + n * C:1 + (n + 1) * C],
                   start=True, stop=False)
                mm_q(pot[:, h * 64:(h + 1) * 64], vns[qp, qf],
                     cx["attnTv"][(h % 2) * 64:(h % 2) * 64 + 64, h // 2,
                                  n * C:(n + 1) * C],
                     start=False, stop=True)
            nc.vector.scalar_tensor_tensor(
                cx["oTv"][:, :, n * C:(n + 1) * C],
                cx["gplv"][:, :, n * C:(n + 1) * C], 1.0,
                pot[:].rearrange("p (h t) -> p h t", h=HG),
                op0=OP.mult, op1=OP.mult)
            pS = ps_c.tile([128, 512], F32, tag="ph2", name="pS")
            for h in range(HG):
                qf = slice((h // 2) * 128, (h // 2) * 128 + 128)
                vsrc = vns if (h % 2) == par else vnsD
                mm_q(pS[:, h * HD:(h + 1) * HD],
                     cx["wkdwnv"][psl, tt, h * HD:(h + 1) * HD],
                     vsrc[psl, qf])
            sscr = vn_pool.tile([128, 512], F32, tag="sscr", name="sscr")
            gam_ = cx["gam"]
            gcol = bass.AP(gam_[:].tensor, gam_[:].offset + n,
                           [[HG * NCHS, 128], [NCHS, HG], [0, HD]])
            nc.vector.tensor_tensor(
                sscr[:].rearrange("p (h e) -> p h e", h=HG),
                S32[:].rearrange("p (h e) -> p h e", h=HG),
                gcol, op=OP.mult)
            nc.vector.tensor_add(S32[:], sscr[:], pS[:])
            nc.scalar.copy(Sbf[:], S32[:])

        def emit_outproj(cx):
            t0_ = cx["t0"]
            for dt_ in range(8):
                pop = ps_a.tile([128, 512], F32, tag="vps", name="pop")
                for h in range(HG):
                    mm(pop[:], wout_v[:, h, dt_ * 128:(dt_ + 1) * 128],
                       cx["oTv"][:, h, :], start=(h == 0), stop=(h == 3))
                ob = os_pool.tile([128, 512], BF16, tag="ob", name="ob")
                nc.vector.tensor_copy(ob[:], pop[:])
                nc.sync.dma_start(outp[dt_, :, t0_:t0_ + TSEG], ob[:])

        prev = None
        for s in range(NSEG):
            t0 = s * TSEG
            # ============ loads ============
            xT = xT_pool.tile([128, 8 * (TSEG + 1)], BF16, tag="xT")
            xTv = xT[:].rearrange("p (k t) -> p k t", k=8)
            nc.sync.dma_start(
                xTv[:],
                xth[:, :, t0:t0 + TSEG + 1].rearrange("k p t -> p k t"))
            xn = xn_pool.tile([128, NTILE * GC], BF16, tag="xn")
            xnv = xn[:].rearrange("p (t n) -> p t n", t=NTILE)
            nc.sync.dma_start(
                xnv[:],
                xnh[1 + t0:1 + t0 + TSEG, :].rearrange("(t p) c -> p t c",
                                                       p=128))
            # shifted x (natural): same HBM tensor, one-token-earlier window
            xs = xs_pool.tile([128, NTILE * GC], BF16, tag="xs")
            xsv = xs[:].rearrange("p (t n) -> p t n", t=NTILE)
            nc.sync.dma_start(
                xsv[:],
                xnh[t0:t0 + TSEG, :].rearrange("(t p) c -> p t c", p=128))

            # per-seg tensors
            rpbf = tr_pool.tile([HG * NBF, TSEG], BF16, tag="rpbf")
            rpf32a = tr_pool.tile([128, TSEG], F32, tag="rpf32a")
            rpf32b = tr_pool.tile([128, TSEG], F32, tag="rpf32b")

            def rpf32_rows(h, col0, ncols):
                t_ = rpf32a if h < 2 else rpf32b
                r0 = (h % 2) * 64
                return t_[r0:r0 + 2, col0:col0 + ncols]
            vnat = ph2_pool.tile([128, NTILE * GC], BF16, tag="vnat")
            vnatv = vnat[:].rearrange("p (t n) -> p t n", t=NTILE)
            wkbn = ph2_pool.tile([128, NTILE * GC], BF16, tag="wkbn")
            wkbnv = wkbn[:].rearrange("p (t n) -> p t n", t=NTILE)
            wkdwn = ph2_pool.tile([128, NTILE * GC], BF16, tag="wkdwn")
            wkdwnv = wkdwn[:].rearrange("p (t n) -> p t n", t=NTILE)
            gpl = ph2_pool.tile([128, HG * TSEG], BF16, tag="gpl")
            gplv = gpl[:].rearrange("p (h t) -> p h t", h=HG)
            attnT = ph2_pool.tile([128, (HG // 2) * TSEG], BF16, tag="attnT")
            attnTv = attnT[:].rearrange("p (r n) -> p r n", r=HG // 2)
            ATd = ph2_pool.tile([128, HG * TSEG], BF16, tag="ATd")
            ATdv = ATd[:].rearrange("p (h t) -> p h t", h=HG)
            wcdT = ph2_pool.tile([128, HG * TSEG], BF16, tag="wcdT")
            wcdTv = wcdT[:].rearrange("p (h t) -> p h t", h=HG)
            oT = ph2_pool.tile([128, HG * TSEG], BF16, tag="oT")
            oTv = oT[:].rearrange("p (h t) -> p h t", h=HG)
            gam = tr_pool.tile([128, HG * NCHS], F32, tag="gam")

            ph2_q = list(range(NCHS)) if prev is not None else []

            def drip(k=1):
                for _ in range(min(k, len(ph2_q))):
                    emit_ph2_chunk(prev, ph2_q.pop(0))

            # ============ per token-tile: projections + scalar bundle ======
            for tt in range(NTILE):
                psv = ps_a.tile([128, GC], F32, tag="vps")
                pss = ps_d.tile([128, 12], F32, tag="small")
                for kb in range(8):
                    xtt = xTv[:, kb, 1 + tt * 128:1 + (tt + 1) * 128]
                    mm(psv[:], xtt, wcat_v[:, kb, :],
                       start=(kb == 0), stop=(kb == 7))
                for kb in range(8):
                    xtt = xTv[:, kb, 1 + tt * 128:1 + (tt + 1) * 128]
                    mm(pss[:], xtt, wsml_v[:, kb, :],
                       start=(kb == 0), stop=(kb == 7))

                bun = bun_pool.tile([128, HG * NQ], F32, tag="bun")
                bv = bun[:].rearrange("p (h q) -> p h q", h=HG)
                scr = bun_pool.tile([128, 24], F32, tag="scr")
                sq = bun_pool.tile([128, 128], F32, tag="sq")
                # norms -> f (from x) and f_shift (from xs, same pipeline)
                for h in range(HG):
                    nc.scalar.activation(sq[:], xnv[:, tt, h * HD:(h + 1) * HD],
                                         AF.Square, accum_out=scr[:, h:h + 1])
                for h in range(HG):
                    nc.scalar.activation(sq[:], xsv[:, tt, h * HD:(h + 1) * HD],
                                         AF.Square,
                                         accum_out=scr[:, 4 + h:5 + h])
                nc.vector.tensor_scalar_max(scr[:, 8:16], scr[:, 0:8], 1e-24)
                nc.scalar.activation(scr[:, 16:24], scr[:, 8:16], AF.Ln)
                nc.scalar.activation(bv[:, :, QF], scr[:, 16:20], AF.Exp,
                                     scale=-0.5)
                nc.scalar.activation(bv[:, :, QFSH], scr[:, 20:24], AF.Exp,
                                     scale=-0.5)
                # sigmoids
                sg = bun_pool.tile([128, 8], F32, tag="sg")
                nc.scalar.activation(sg[:, 0:4], pss[:, 0:4], AF.Exp,
                                     scale=-1.0)
                nc.scalar.activation(sg[:, 4:8], pss[:, 8:12], AF.Exp,
                                     scale=-1.0)
                nc.vector.tensor_scalar_add(sg[:, 0:8], sg[:, 0:8], 1.0)
                nc.vector.reciprocal(bv[:, :, QBETA], sg[:, 0:4])
                nc.vector.reciprocal(bv[:, :, QG], sg[:, 4:8])
                # decay
                nc.vector.tensor_add(scr[:, 12:16], pss[:, 4:8], dtb_sb[:])
                nc.scalar.activation(scr[:, 16:20], scr[:, 12:16], AF.Exp)
                nc.scalar.activation(scr[:, 16:20], scr[:, 16:20], AF.Ln,
                                     bias=1.0)
                nc.vector.tensor_mul(scr[:, 20:24], scr[:, 16:20], aneg_sb[:])
                # within-chunk cumulative decay
                psc = ps_d.tile([128, 4], F32, tag="small")
                mm(psc[:], triucum_sb[:], scr[:, 20:24])
                nc.scalar.copy(bv[:, :, QDEC], psc[:])
                psl = ps_d.tile([128, 4], F32, tag="small")
                mm(psl[:], e64sel_sb[:], bv[:, :, QDEC])
                nc.vector.tensor_sub(scr[:, 0:4], psl[:], bv[:, :, QDEC])
                nc.scalar.activation(scr[:, 4:8], scr[:, 0:4], AF.Exp)  # dw
                nc.scalar.activation(bv[:, :, QEDEC], bv[:, :, QDEC], AF.Exp)
                nc.scalar.activation(bv[:, :, QEDEC2], bv[:, :, QDEC], AF.Exp)
                nc.scalar.activation(bv[:, :, QEDECI], bv[:, :, QDEC], AF.Exp,
                                     scale=-1.0)
                nc.vector.tensor_mul(bv[:, :, QRA], bv[:, :, QF],
                                     bv[:, :, QEDEC])
                # fold f*edec into the gate: the rk-side per-token factor is
                # applied to pot's output columns via gpl instead of to xT
                nc.vector.tensor_mul(bv[:, :, QG], bv[:, :, QG],
                                     bv[:, :, QRA])
                nc.vector.tensor_mul(scr[:, 8:12], bv[:, :, QFSH],
                                     bv[:, :, QBETA])
                nc.vector.tensor_mul(bv[:, :, QFB], scr[:, 8:12],
                                     bv[:, :, QEDEC])
                nc.vector.tensor_mul(bv[:, :, QWB], bv[:, :, QFSH],
                                     bv[:, :, QEDECI])
                nc.vector.tensor_mul(bv[:, :, QFDW], bv[:, :, QFSH],
                                     scr[:, 4:8])
                # transposed per-token scalars
                bsh = bun_pool.tile([128, HG * NBF], BF16, tag="bsh")
                nc.vector.tensor_copy(
                    bsh[:].rearrange("p (h q) -> p h q", h=HG),
                    bv[:, :, 0:NBF])
                ptb = ps_d.tile([HG * NBF, 128], BF16, tag="small")
                nc.tensor.transpose(ptb[:], bsh[:], identbf_sb[:])
                nc.scalar.copy(rpbf[:, tt * 128:(tt + 1) * 128], ptb[:])
                for h in range(HG):
                    ptf = ps_d.tile([2, 128], F32, tag="small")
                    nc.tensor.transpose(
                        ptf[:], bun[:, h * NQ + QDEC:h * NQ + QDEC + 2],
                        identf_sb[:])
                    nc.scalar.copy(rpf32_rows(h, tt * 128, 128), ptf[:])
                # v' = beta * v  (natural, bf16)
                nc.vector.scalar_tensor_tensor(
                    vnatv[:, tt, :].rearrange("p (h e) -> p h e", h=HG),
                    psv[:].rearrange("p (h e) -> p h e", h=HG), 1.0,
                    bv[:, :, QBETA].broadcast_to((128, HG, HD)),
                    op0=OP.mult, op1=OP.mult)
                # shifted-key natural tensors
                nc.vector.scalar_tensor_tensor(
                    wkbnv[:, tt, :].rearrange("p (h e) -> p h e", h=HG),
                    xsv[:, tt, :].rearrange("p (h e) -> p h e", h=HG), 1.0,
                    bv[:, :, QFB].broadcast_to((128, HG, HD)),
                    op0=OP.mult, op1=OP.mult)
                nc.vector.scalar_tensor_tensor(
                    wkdwnv[:, tt, :].rearrange("p (h e) -> p h e", h=HG),
                    xsv[:, tt, :].rearrange("p (h e) -> p h e", h=HG), 1.0,
                    bv[:, :, QFDW].broadcast_to((128, HG, HD)),
                    op0=OP.mult, op1=OP.mult)
                if tt > 0:
                    drip()

            if stage < 2:
                continue
            # ============ per head: T-side tiles + gamma ============
            wkA_l, wkB_l = [], []
            for h in range(HG):
                wkA = ph1_pool.tile([128, TSEG], BF16, tag=f"wkA{h}")
                wkB = ph1_pool.tile([128, TSEG], BF16, tag=f"wkB{h}")
                wkA_l.append(wkA); wkB_l.append(wkB)

                rp16 = rpbf[0:16, :]
                pa = ps_b.tile([128, TSEG], F32, tag="prod")
                selbf_mm(pa[:], h * NBF + QFB, rp16)
                nc.vector.scalar_tensor_tensor(
                    wkA[:], xTv[:, h, 0:TSEG], 1.0, pa[:],
                    op0=OP.mult, op1=OP.mult)
                pb = ps_b.tile([128, TSEG], F32, tag="prod")
                selbf_mm(pb[:], h * NBF + QWB, rp16)
                nc.vector.scalar_tensor_tensor(
                    wkB[:], xTv[:, h, 0:TSEG], 1.0, pb[:],
                    op0=OP.mult, op1=OP.mult)
                pg = ps_b.tile([128, TSEG], F32, tag="prod")
                selbf_mm(pg[:], h * NBF + QG, rp16)
                nc.scalar.copy(gplv[:, h, :], pg[:])
                # gamma = exp(dec at chunk end), broadcast to all partitions
                pgm = ps_d.tile([128, NCHS], F32, tag="small")
                rt = rpf32a if h < 2 else rpf32b
                r0 = (h % 2) * 64
                gsel2 = bass.AP(rt[:].tensor,
                                rt[:].offset + (h % 2) * 64 * TSEG + 63,
                                [[TSEG, 2], [C, NCHS]])
                mm(pgm[:], sel2f_sb[r0:r0 + 2, 128:256], gsel2)
                nc.scalar.copy(gam[:, h * NCHS:(h + 1) * NCHS], pgm[:])

            if stage < 3:
                continue
            # ===== phase-1: products, masks, 3-factor inverse =====
            # A.T = ((I-M)(I+M^2)(I+M^4)).T, error O(M^8); the two head
            # pairs (pr) are interleaved stage-by-stage so PE never waits
            # on a PSUM drain (the other pair's matmuls cover it).
            def chunk_mms(out_ps, lh, rh, pr):
                for hh in range(2):
                    sl = slice(hh * 64, (hh + 1) * 64)
                    for n in range(NCHS):
                        csl = slice(n * C, (n + 1) * C)
                        mm(out_ps[sl, csl], lh[sl, csl], rh[sl, csl])

            Msb_, MTsb_, ImM_, P1r_, P1i_, Q1r_, G0_, G1_ = ({} for _ in
                                                             range(8))
            for pr in range(2):
                pp1 = ps_b.tile([128, 512], F32, tag="prod")
                for hh in range(2):
                    h = pr * 2 + hh
                    sl = slice(hh * 64, (hh + 1) * 64)
                    for n in range(NCHS):
                        csl = slice(n * C, (n + 1) * C)
                        mm(pp1[sl, csl], wkA_l[h][:, csl], wkB_l[h][:, csl])
                Msb_[pr] = ph1_pool.tile([128, 512], BF16, tag=f"Msb{pr}",
                               name=f"Msb{pr}")
                nc.vector.scalar_tensor_tensor(Msb_[pr][:], pp1[:], 1.0,
                                               mSL_sb[:],
                                               op0=OP.mult, op1=OP.mult)
            for pr in range(2):
                pp1t = ps_b.tile([128, 512], F32, tag="prod")
                for hh in range(2):
                    h = pr * 2 + hh
                    sl = slice(hh * 64, (hh + 1) * 64)
                    for n in range(NCHS):
                        csl = slice(n * C, (n + 1) * C)
                        mm(pp1t[sl, csl], wkB_l[h][:, csl], wkA_l[h][:, csl])
                MTsb_[pr] = ph1_pool.tile([128, 512], BF16, tag=f"MTsb{pr}",
                                name=f"MTsb{pr}")
                nc.vector.scalar_tensor_tensor(MTsb_[pr][:], pp1t[:], 1.0,
                                               mSU_sb[:],
                                               op0=OP.mult, op1=OP.mult)
                ImM_[pr] = ph1_pool.tile([128, 512], BF16, tag=f"ImM{pr}",
                               name=f"ImM{pr}")
                nc.vector.scalar_tensor_tensor(ImM_[pr][:], Msb_[pr][:], -1.0,
                                               i2x8_sb[:],
                                               op0=OP.mult, op1=OP.add)
            drip()
            for pr in range(2):
                pp2 = ps_b.tile([128, 512], F32, tag="prod")
                for hh in range(2):
                    h = pr * 2 + hh
                    sl = slice(hh * 64, (hh + 1) * 64)
                    for n in range(NCHS):
                        csl = slice(n * C, (n + 1) * C)
                        mm(pp2[sl, csl], wkB_l[h][:, csl],
                           xTv[:, h, 1 + n * C:1 + (n + 1) * C])
                nc.vector.scalar_tensor_tensor(attnTv[:, pr, :], pp2[:], 1.0,
                                               mUI_sb[:],
                                               op0=OP.mult, op1=OP.mult)
            drip()
            for pr in range(2):
                pP1 = ps_b.tile([128, 512], F32, tag="prod")
                chunk_mms(pP1, MTsb_[pr][:], Msb_[pr][:], pr)
                P1r_[pr] = ph1_pool.tile([128, 512], BF16, tag=f"P1r{pr}",
                               name=f"P1r{pr}")
                P1i_[pr] = ph1_pool.tile([128, 512], BF16, tag=f"P1i{pr}",
                               name=f"P1i{pr}")
                nc.scalar.copy(P1r_[pr][:], pP1[:])
                nc.vector.scalar_tensor_tensor(P1i_[pr][:], pP1[:], 1.0,
                                               i2x8_sb[:],
                                               op0=OP.mult, op1=OP.add)
            drip()
            for pr in range(2):
                pQ1 = ps_b.tile([128, 512], F32, tag="prod")
                chunk_mms(pQ1, Msb_[pr][:], MTsb_[pr][:], pr)
                Q1r_[pr] = ph1_pool.tile([128, 512], BF16, tag=f"Q1r{pr}",
                               name=f"Q1r{pr}")
                nc.scalar.copy(Q1r_[pr][:], pQ1[:])
            drip()
            P2r_, P2i_, Q2r_, G2_ = {}, {}, {}, {}
            for pr in range(2):
                pP2 = ps_b.tile([128, 512], F32, tag="prod")
                chunk_mms(pP2, Q1r_[pr][:], P1r_[pr][:], pr)
                P2r_[pr] = ph1_pool.tile([128, 512], BF16, tag=f"P2r{pr}",
                                         name=f"P2r{pr}")
                P2i_[pr] = ph1_pool.tile([128, 512], BF16, tag=f"P2i{pr}",
                                         name=f"P2i{pr}")
                nc.scalar.copy(P2r_[pr][:], pP2[:])
                nc.vector.scalar_tensor_tensor(P2i_[pr][:], pP2[:], 1.0,
                                               i2x8_sb[:],
                                               op0=OP.mult, op1=OP.add)
            drip()
            for pr in range(2):
                pQ2 = ps_b.tile([128, 512], F32, tag="prod")
                chunk_mms(pQ2, P1r_[pr][:], Q1r_[pr][:], pr)
                Q2r_[pr] = ph1_pool.tile([128, 512], BF16, tag=f"Q2r{pr}",
                                         name=f"Q2r{pr}")
                nc.scalar.copy(Q2r_[pr][:], pQ2[:])
            for pr in range(2):
                pQ3 = ps_b.tile([128, 512], F32, tag="prod")
                chunk_mms(pQ3, P2r_[pr][:], Q2r_[pr][:], pr)
                G0_[pr] = ph1_pool.tile([128, 512], BF16, tag=f"G0{pr}",
                              name=f"G0{pr}")
                nc.vector.scalar_tensor_tensor(G0_[pr][:], pQ3[:], 1.0,
                                               i2x8_sb[:],
                                               op0=OP.mult, op1=OP.add)
            for pr in range(2):
                pG1 = ps_b.tile([128, 512], F32, tag="prod")
                chunk_mms(pG1, P2i_[pr][:], G0_[pr][:], pr)
                G1_[pr] = ph1_pool.tile([128, 512], BF16, tag=f"G1{pr}",
                              name=f"G1{pr}")
                nc.scalar.copy(G1_[pr][:], pG1[:])
            for pr in range(2):
                pG2 = ps_b.tile([128, 512], F32, tag="prod")
                chunk_mms(pG2, P1i_[pr][:], G1_[pr][:], pr)
                G2_[pr] = ph1_pool.tile([128, 512], BF16, tag=f"G2{pr}",
                                        name=f"G2{pr}")
                nc.scalar.copy(G2_[pr][:], pG2[:])
            for pr in range(2):
                pAT = ps_b.tile([128, 512], F32, tag="prod")
                chunk_mms(pAT, ImM_[pr][:], G2_[pr][:], pr)
                # duplicate each chunk's AT at both partition parities
                for hh in range(2):
                    h = pr * 2 + hh
                    for par in range(2):
                        nc.scalar.copy(
                            ATdv[par * 64:(par + 1) * 64, h, :].rearrange(
                                "p (n c) -> p n c", c=C)[:, par::2, :],
                            pAT[hh * 64:(hh + 1) * 64, :].rearrange(
                                "p (n c) -> p n c", c=C)[:, par::2, :])

            if stage < 4:
                continue
            # wk_cumdecay.T = -(A @ wkb')^T per (head, chunk)
            for h in range(HG):
                pwc = ps_b.tile([128, 512], F32, tag="prod")
                for n in range(NCHS):
                    mm_q(pwc[:, n * C:(n + 1) * C],
                       wkbnv[(n % 2) * 64:(n % 2) * 64 + 64, n // 2,
                             h * HD:(h + 1) * HD],
                       ATdv[(n % 2) * 64:(n % 2) * 64 + 64, h,
                            n * C:(n + 1) * C])
                nc.vector.tensor_scalar_mul(wcdTv[:, h, :], pwc[:], -1.0)

            if stage < 4.5:
                continue
            # flush any chunks of seg s-1 not yet dripped, then its outproj
            drip(NCHS)
            if prev is not None:
                emit_outproj(prev)
            prev = dict(xTv=xTv, vnatv=vnatv, wkbnv=wkbnv, wkdwnv=wkdwnv,
                        attnTv=attnTv, ATdv=ATdv, wcdTv=wcdTv, oTv=oTv,
                        gplv=gplv, gam=gam, t0=t0)

        # epilogue: last segment's recurrence + output projection
        if prev is not None:
            for n in range(NCHS):
                emit_ph2_chunk(prev, n)
            emit_outproj(prev)

    return nc


def _merge_waits(waits):
    """Merge duplicate-sem waits keeping the max threshold (sem-ge modes)."""
    best, order = {}, []
    for w in waits:
        k = getattr(w, "ant_name", None) or str(getattr(w, "id", ""))
        if k not in best:
            best[k] = w
            order.append(k)
        elif (getattr(w, "wait_value", 0) or 0) > (getattr(best[k], "wait_value", 0) or 0):
            best[k] = w
    return [best[k] for k in order]


def _patch_commit_for_wait_caps(tc, nc, cap=1):
    """Wrap TileContext._commit_instruction: instructions whose wait list
    exceeds the ISA sync-slot budget get standalone EventSemaphore carriers
    emitted immediately before them on the same engine."""
    orig = tc._commit_instruction

    def patched(inst, lazy_reg_writes=True):
        si = getattr(inst, "sync_info", None)
        eng = getattr(inst, "engine", None)
        if si is not None and si.on_wait and eng is not None:
            w = _merge_waits(list(si.on_wait))
            if len(w) > cap:
                keep, excess = w[:cap], w[cap:]
                for ww in excess:
                    ev = mybir.InstDrain(
                        name=nc.get_next_instruction_name(),
                        ins=[], outs=[],
                        sync_info=mybir.SyncInfo(on_wait=[ww], on_update=[]))
                    ev.engine = eng
                    orig(ev, lazy_reg_writes=False)
                w = keep
            if len(w) != len(si.on_wait):
                inst.sync_info = mybir.SyncInfo(
                    on_wait=w, on_update=list(si.on_update or []))
        return orig(inst, lazy_reg_writes)

    tc._commit_instruction = patched

    orig_dab = tc._drain_and_barrier

    def patched_dab(tick_clock, wait_clock):
        from concourse.tile import ScopedClock
        d = nc.sync.drain()
        wait_clock.add_sem_waits(
            d.ins, ScopedClock({None: tick_clock.global_clock}))
        si = d.ins.sync_info
        if si is not None and si.on_wait and len(si.on_wait) > 1:
            extra = list(si.on_wait[1:])
            d.ins.sync_info = mybir.SyncInfo(
                on_wait=[si.on_wait[0]],
                on_update=list(si.on_update or []))
            for w in extra:
                d2 = nc.sync.drain()
                d2.ins.sync_info = mybir.SyncInfo(on_wait=[w], on_update=[])
        nc.all_engine_barrier()
        popped = nc._tile_sem_poison_stack.pop()
        assert popped is tc._sem_poison
        nc.clear_and_free_semaphores(list(tc.sems.allocated().values()))
        nc.all_engine_barrier()

    tc._drain_and_barrier = patched_dab


# ---------------- host side ----------------

def _prep_core_inputs(x_b, g, W_write, W_gate, W_out, W_beta, W_alpha,
                      dt_bias, A_log, Ttot):
    perm = np.arange(D) if g == 0 else np.concatenate(
        [np.arange(GC, 2 * GC), np.arange(0, GC)])
    xr = x_b[:, perm]
    hsl = slice(g * HG, (g + 1) * HG)
    Ww = W_write[g * GC:(g + 1) * GC, :][:, perm]
    Wsml = np.concatenate([W_beta[hsl], W_alpha[hsl], W_gate[hsl]], 0)[:, perm]
    Wo = W_out[:, g * GC:(g + 1) * GC]

    wcat_np = np.ascontiguousarray(
        Ww.T.reshape(8, 128, GC).transpose(1, 0, 2)).astype(ml_dtypes.bfloat16)
    wsml_np = np.ascontiguousarray(
        Wsml.T.reshape(8, 128, 12).transpose(1, 0, 2)).astype(ml_dtypes.bfloat16)
    wout_np = np.ascontiguousarray(
        Wo.T.reshape(HG, 128, 1024).transpose(1, 0, 2)).astype(ml_dtypes.bfloat16)
    dtb_np = np.broadcast_to(dt_bias[hsl], (128, HG)).astype(np.float32)
    aneg_np = np.broadcast_to(-np.exp(A_log[hsl]), (128, HG)).astype(np.float32)
    xb = xr[:Ttot].astype(ml_dtypes.bfloat16)
    xthn = np.zeros((8, 128, Ttot + 1), ml_dtypes.bfloat16)
    xthn[:, :, 1:] = np.ascontiguousarray(xb.T).reshape(8, 128, Ttot)
    xnhp = np.zeros((Ttot + 1, GC), ml_dtypes.bfloat16)
    xnhp[1:] = xb[:, 0:GC]
    return {
        "xth": xthn,
        "xnh": xnhp,
        "wcat": wcat_np, "wsml": wsml_np, "wout": wout_np,
        "dtb": np.ascontiguousarray(dtb_np),
        "aneg": np.ascontiguousarray(aneg_np),
    }


_NC_CACHE = {}


def kernel(x, W_write, W_gate, W_out, W_beta, W_alpha, dt_bias, A_log,
           _trace=False):
    from concourse.bass_utils import run_bass_kernel_spmd

    x = np.asarray(x)
    Bn, Tn, Dm = x.shape
    if Tn not in _NC_CACHE:
        _NC_CACHE[Tn] = build_nc(Ttot=Tn)
    nc = _NC_CACHE[Tn]

    in_maps = []
    for core in range(NCORES):
        b, g = core // 2, core % 2
        in_maps.append(_prep_core_inputs(
            np.asarray(x[b]), g, np.asarray(W_write), np.asarray(W_gate),
            np.asarray(W_out), np.asarray(W_beta), np.asarray(W_alpha),
            np.asarray(dt_bias), np.asarray(A_log), Tn))

    res = run_bass_kernel_spmd(nc, in_maps, core_ids=list(range(NCORES)),
                               trace=_trace)
    out = np.empty((Bn, Tn, Dm), np.float32)
    for b in range(Bn):
        p0 = res.results[2 * b]["outp"].reshape(Dm, Tn).astype(np.float32)
        p1 = res.results[2 * b + 1]["outp"].reshape(Dm, Tn).astype(np.float32)
        out[b] = x[b] + p0.T + p1.T
    if _trace:
        kernel._last_results = res
    return out



# revision 27
# speedup vs baseline: 1.2231x; 1.0505x over previous
"""Trainium2 Bass kernel for the DeltaHebbian (gated delta-rule) block.

Sharding: 8 cores = 4 batches x 2 head-groups (4 heads each). Each core gets
its batch's x with columns rotated so its head-group occupies cols 0:512, and
computes partial_out.T = (gated_o @ W_out_slice.T).T.  Host sums the two
partials per batch and adds x.

Per-core algorithm (chunked delta rule, CHUNK=64):
  phase 1 (token-parallel): projections, key normalization, per-chunk decay
  cumsums, masked key-product matrices M / M.T / attn.T, and the UT-transform
  inverse A.T = ((I+M)^-1).T via the telescoping factorization
  (I-M)(I+M^2)(I+M^4)(I+M^8)  (exact to ~4e-5 on this data: |M^16| ~ 5e-5).
  phase 2 (sequential over chunks, 4 heads interleaved): the state recurrence.
"""

import sys

for _p in ("/opt/trn_rl_repo",):
    if _p not in sys.path:
        sys.path.append(_p)

from contextlib import ExitStack

import numpy as np
import ml_dtypes

import concourse.bass as bass
import concourse.mybir as mybir
import concourse.tile as tile

F32 = mybir.dt.float32
BF16 = mybir.dt.bfloat16
OP = mybir.AluOpType
AF = mybir.ActivationFunctionType

# problem constants
B, T, D = 4, 8192, 1024
HD = 128          # head dim
C = 64            # chunk length
HG = 4            # heads per core
GC = HG * HD      # 512 group columns
NCORES = 8
NQ = 12           # bundle quantities per head
# bundle column indices (per head, stride NQ); cols 0..3 are the bf16
# plane factors (transposed then token-broadcast)
(QRA, QFB, QWB, QG, QF, QFSH, QEDEC, QEDECI, QDEC, QEDEC2, QBETA,
 QFDW) = range(12)
NBF = 4           # bf16 transposed rows per head: cols 0..3


def _consts():
    ii = np.arange(128)
    jj = np.arange(512)
    pi = ii[:, None] % 64
    qi = jj[None, :] % 64
    c = {}
    c["i2x8"] = (pi == qi).astype(np.float32)
    c["mSL"] = (pi > qi).astype(np.float32)      # keep i>j   (M)
    c["mSU"] = (qi > pi).astype(np.float32)      # keep j>i   (M.T)
    c["mUI"] = (qi >= pi).astype(np.float32)     # keep i>=j  (attn.T)
    k = np.arange(128)
    m = np.arange(128)
    same = (k[:, None] // 64) == (m[None, :] // 64)
    c["triucum"] = (same & ((k[:, None] % 64) <= (m[None, :] % 64))).astype(np.float32)
    c["e64sel"] = (k[:, None] == (m[None, :] // 64) * 64 + 63).astype(np.float32)
    c["identbf"] = np.eye(128).astype(ml_dtypes.bfloat16)
    c["identf"] = np.eye(128).astype(np.float32)
    c["ones4"] = np.ones((128, 4), np.float32)
    c["ones1"] = np.ones((1, 128), np.float32)
    sel4 = np.zeros((4, 4 * 128), np.float32)
    for h4 in range(4):
        sel4[h4, h4 * 128:(h4 + 1) * 128] = 1.0
    c["sel4"] = sel4
    sh1 = (k[:, None] == m[None, :] - 1).astype(np.float32)   # out[m]=in[m-1]
    c["sh1f"] = sh1
    c["sh1bf"] = sh1.astype(ml_dtypes.bfloat16)
    s127 = np.zeros((128, 128), np.float32)   # out row0 = in row127, rest += 0
    s127[127, 0] = 1.0
    c["sel127f"] = s127
    c["sel127bf"] = s127.astype(ml_dtypes.bfloat16)
    # bf16 row selectors: target t -> (16, 128) block with row t all-ones
    selbf = np.zeros((16, 16 * 128), np.float32)
    for t in range(16):
        selbf[t, t * 128:(t + 1) * 128] = 1.0
    c["selbf"] = selbf.astype(ml_dtypes.bfloat16)
    sel2 = np.zeros((128, 2 * 128), np.float32)  # [dec-sel | edec-sel] at rows 0/64
    for hh in (0, 64):
        sel2[hh + 0, 0:128] = 1.0
        sel2[hh + 1, 128:256] = 1.0
    c["sel2f"] = sel2
    return c


def build_nc(Ttot=T, TSEG=512, stage=5):
    assert Ttot % TSEG == 0 and TSEG == 512
    NSEG = Ttot // TSEG
    NTILE = TSEG // 128
    NCHS = TSEG // C

    nc = bass.Bass()
    xth = nc.dram_tensor("xth", (8, 128, Ttot + 1), BF16, kind="ExternalInput")
    xnh = nc.dram_tensor("xnh", (Ttot + 1, GC), BF16, kind="ExternalInput")
    wcat = nc.dram_tensor("wcat", (128, 8, GC), BF16, kind="ExternalInput")
    wsml = nc.dram_tensor("wsml", (128, 8, 12), BF16, kind="ExternalInput")
    wout = nc.dram_tensor("wout", (128, HG, 1024), BF16, kind="ExternalInput")
    dtb = nc.dram_tensor("dtb", (128, 4), F32, kind="ExternalInput")
    aneg = nc.dram_tensor("aneg", (128, 4), F32, kind="ExternalInput")
    outp = nc.dram_tensor("outp", (8, 128, Ttot), BF16, kind="ExternalOutput")

    cst = _consts()
    dr = {k: nc.inline_tensor(v, name=f"c_{k}") for k, v in cst.items()}

    with tile.TileContext(nc) as tc, ExitStack() as ctx:
        _patch_commit_for_wait_caps(tc, nc)
        # ---- persistent SBUF ----
        cp = ctx.enter_context(tc.tile_pool(name="consts", bufs=1))
        wcat_sb = cp.tile([128, 8 * GC], BF16, tag="wcat")
        wsml_sb = cp.tile([128, 8 * 12], BF16, tag="wsml")
        wout_sb = cp.tile([128, HG * 1024], BF16, tag="wout")
        dtb_sb = cp.tile([128, 4], F32, tag="dtb")
        aneg_sb = cp.tile([128, 4], F32, tag="aneg")
        i2x8_sb = cp.tile([128, 512], F32, tag="i2x8")
        mSL_sb = cp.tile([128, 512], F32, tag="mSL")
        mSU_sb = cp.tile([128, 512], F32, tag="mSU")
        mUI_sb = cp.tile([128, 512], F32, tag="mUI")
        triucum_sb = cp.tile([128, 128], F32, tag="triucum")
        e64sel_sb = cp.tile([128, 128], F32, tag="e64sel")
        identbf_sb = cp.tile([128, 128], BF16, tag="identbf")
        identf_sb = cp.tile([128, 128], F32, tag="identf")
        ones4_sb = cp.tile([128, 4], F32, tag="ones4")
        ones1_sb = cp.tile([1, 128], F32, tag="ones1")
        sel4_sb = cp.tile([4, 4 * 128], F32, tag="sel4")
        sh1f_sb = cp.tile([128, 128], F32, tag="sh1f")
        sh1bf_sb = cp.tile([128, 128], BF16, tag="sh1bf")
        sel127f_sb = cp.tile([128, 128], F32, tag="sel127f")
        sel127bf_sb = cp.tile([128, 128], BF16, tag="sel127bf")
        selbf_sb = cp.tile([16, 16 * 128], BF16, tag="selbf")
        sel2f_sb = cp.tile([128, 2 * 128], F32, tag="sel2f")
        S32 = cp.tile([128, HG * HD], F32, tag="S32")
        Sbf = cp.tile([128, HG * HD], BF16, tag="Sbf")

        for nm, t_ in (("i2x8", i2x8_sb), ("mSL", mSL_sb), ("mSU", mSU_sb),
                       ("mUI", mUI_sb), ("triucum", triucum_sb),
                       ("e64sel", e64sel_sb), ("identbf", identbf_sb),
                       ("identf", identf_sb), ("ones4", ones4_sb),
                       ("ones1", ones1_sb), ("sel4", sel4_sb),
                       ("sh1f", sh1f_sb), ("sh1bf", sh1bf_sb),
                       ("sel127f", sel127f_sb), ("sel127bf", sel127bf_sb),
                       ("selbf", selbf_sb), ("sel2f", sel2f_sb)):
            nc.sync.dma_start(t_[:], dr[nm][:])
        nc.sync.dma_start(wcat_sb[:].rearrange("p (k n) -> p k n", k=8), wcat[:])
        nc.sync.dma_start(wsml_sb[:].rearrange("p (k n) -> p k n", k=8), wsml[:])
        nc.sync.dma_start(wout_sb[:].rearrange("p (h n) -> p h n", h=HG), wout[:])
        nc.sync.dma_start(dtb_sb[:], dtb[:])
        nc.sync.dma_start(aneg_sb[:], aneg[:])
        nc.gpsimd.memset(S32[:], 0.0)
        nc.gpsimd.memset(Sbf[:], 0.0)

        # ---- pools ----
        xT_pool = ctx.enter_context(tc.tile_pool(name="xT", bufs=2))
        xn_pool = ctx.enter_context(tc.tile_pool(name="xn", bufs=2))
        ph1_pool = ctx.enter_context(tc.tile_pool(name="ph1", bufs=1))
        xs_pool = ctx.enter_context(tc.tile_pool(name="xs", bufs=2))
        ph2_pool = ctx.enter_context(tc.tile_pool(name="ph2", bufs=2))
        bun_pool = ctx.enter_context(tc.tile_pool(name="bun", bufs=3))
        tr_pool = ctx.enter_context(tc.tile_pool(name="tr", bufs=2))
        vn_pool = ctx.enter_context(tc.tile_pool(name="vn", bufs=3))
        os_pool = ctx.enter_context(tc.tile_pool(name="os", bufs=2))
        bc_pool = ctx.enter_context(tc.tile_pool(name="bc", bufs=2))

        ps_a = ctx.enter_context(tc.tile_pool(name="psA", bufs=2, space="PSUM"))
        ps_b = ctx.enter_context(tc.tile_pool(name="psB", bufs=2, space="PSUM"))
        ps_c = ctx.enter_context(tc.tile_pool(name="psC", bufs=3, space="PSUM"))
        ps_d = ctx.enter_context(tc.tile_pool(name="psD", bufs=1, space="PSUM"))

        def mm(out, lhsT, rhs, start=True, stop=True, tp=None):
            nc.tensor.matmul(out, lhsT, rhs, start=start, stop=stop)

        def mm_q(out, lhsT, rhs, start=True, stop=True):
            # K-operands at partition offset 64 fault at runtime when M=128
            # (full-width row-offset tile); split into two 64-col quadrants.
            if lhsT.base_partition() != 0 and lhsT.free_size() > 64:
                assert lhsT.free_size() == 128
                nc.tensor.matmul(out[0:64, :], lhsT[:, 0:64], rhs,
                                 start=start, stop=stop)
                nc.tensor.matmul(out[64:128, :], lhsT[:, 64:128], rhs,
                                 start=start, stop=stop)
            else:
                nc.tensor.matmul(out, lhsT, rhs, start=start, stop=stop)

        def selbf_mm(out, target, rhs_cols):
            """out[m, t] = rpbf[target, t] broadcast over 128 partitions."""
            mm(out, selbf_sb[:, target * 128:(target + 1) * 128], rhs_cols)

        wcat_v = wcat_sb[:].rearrange("p (k n) -> p k n", k=8)
        wsml_v = wsml_sb[:].rearrange("p (k n) -> p k n", k=8)
        wout_v = wout_sb[:].rearrange("p (h n) -> p h n", h=HG)

        # ---- phase-2 chunk + output projection, deferred one segment ----
        # Segment s's sequential state recurrence is emitted interleaved
        # into segment s+1's token-parallel work so the chunk chain's
        # non-PE latency is covered by phase-1 matmuls.
        def emit_ph2_chunk(cx, n):
            tt, par = n // 2, n % 2
            psl = slice(par * 64, par * 64 + 64)
            pvn = ps_c.tile([128, 256], F32, tag="ph2", name="pvn")
            for h in range(HG):
                qp = slice((h % 2) * 64, (h % 2) * 64 + 64)
                qf = slice((h // 2) * 128, (h // 2) * 128 + 128)
                mm(pvn[qp, qf],
                   cx["ATdv"][psl, h, n * C:(n + 1) * C],
                   cx["vnatv"][psl, tt, h * HD:(h + 1) * HD],
                   start=True, stop=False)
                mm(pvn[qp, qf], cx["wcdTv"][:, h, n * C:(n + 1) * C],
                   Sbf[:, h * HD:(h + 1) * HD],
                   start=False, stop=True)
            vns = vn_pool.tile([128, 256], BF16, tag="vns", name="vns")
            nc.scalar.copy(vns[:], pvn[:])
            vnsD = vn_pool.tile([128, 256], BF16, tag="vnsD", name="vnsD")
            nc.vector.tensor_copy(vnsD[0:64, :], vns[64:128, :])
            nc.vector.tensor_copy(vnsD[64:128, :], vns[0:64, :])
            pot = ps_c.tile([128, 256], F32, tag="ph2", name="pot")
            for h in range(HG):
                qp = slice((h % 2) * 64, (h % 2) * 64 + 64)
                qf = slice((h // 2) * 128, (h // 2) * 128 + 128)
                mm(pot[:, h * 64:(h + 1) * 64],
                   Sbf[:, h * HD:(h + 1) * HD],
                   cx["xTv"][:, h, 1 + n * C:1 + (n + 1) * C],
                   start=True, stop=False)
                mm_q(pot[:, h * 64:(h + 1) * 64], vns[qp, qf],
                     cx["attnTv"][(h % 2) * 64:(h % 2) * 64 + 64, h // 2,
                                  n * C:(n + 1) * C],
                     start=False, stop=True)
            nc.vector.scalar_tensor_tensor(
                cx["oTv"][:, :, n * C:(n + 1) * C],
                cx["gplv"][:, :, n * C:(n + 1) * C], 1.0,
                pot[:].rearrange("p (h t) -> p h t", h=HG),
                op0=OP.mult, op1=OP.mult)
            pS = ps_c.tile([128, 512], F32, tag="ph2", name="pS")
            for h in range(HG):
                qf = slice((h // 2) * 128, (h // 2) * 128 + 128)
                vsrc = vns if (h % 2) == par else vnsD
                mm_q(pS[:, h * HD:(h + 1) * HD],
                     cx["wkdwnv"][psl, tt, h * HD:(h + 1) * HD],
                     vsrc[psl, qf])
            sscr = vn_pool.tile([128, 512], F32, tag="sscr", name="sscr")
            gam_ = cx["gam"]
            gcol = bass.AP(gam_[:].tensor, gam_[:].offset + n,
                           [[HG * NCHS, 128], [NCHS, HG], [0, HD]])
            nc.vector.tensor_tensor(
                sscr[:].rearrange("p (h e) -> p h e", h=HG),
                S32[:].rearrange("p (h e) -> p h e", h=HG),
                gcol, op=OP.mult)
            nc.vector.tensor_add(S32[:], sscr[:], pS[:])
            nc.scalar.copy(Sbf[:], S32[:])

        def emit_outproj(cx):
            t0_ = cx["t0"]
            for dt_ in range(8):
                pop = ps_a.tile([128, 512], F32, tag="vps", name="pop")
                for h in range(HG):
                    mm(pop[:], wout_v[:, h, dt_ * 128:(dt_ + 1) * 128],
                       cx["oTv"][:, h, :], start=(h == 0), stop=(h == 3))
                ob = os_pool.tile([128, 512], BF16, tag="ob", name="ob")
                nc.vector.tensor_copy(ob[:], pop[:])
                nc.sync.dma_start(outp[dt_, :, t0_:t0_ + TSEG], ob[:])

        prev = None
        for s in range(NSEG):
            t0 = s * TSEG
            # ============ loads ============
            xT = xT_pool.tile([128, 8 * (TSEG + 1)], BF16, tag="xT")
            xTv = xT[:].rearrange("p (k t) -> p k t", k=8)
            nc.sync.dma_start(
                xTv[:],
                xth[:, :, t0:t0 + TSEG + 1].rearrange("k p t -> p k t"))
            xn = xn_pool.tile([128, NTILE * GC], BF16, tag="xn")
            xnv = xn[:].rearrange("p (t n) -> p t n", t=NTILE)
            nc.sync.dma_start(
                xnv[:],
                xnh[1 + t0:1 + t0 + TSEG, :].rearrange("(t p) c -> p t c",
                                                       p=128))
            # shifted x (natural): same HBM tensor, one-token-earlier window
            xs = xs_pool.tile([128, NTILE * GC], BF16, tag="xs")
            xsv = xs[:].rearrange("p (t n) -> p t n", t=NTILE)
            nc.sync.dma_start(
                xsv[:],
                xnh[t0:t0 + TSEG, :].rearrange("(t p) c -> p t c", p=128))

            # per-seg tensors
            rpbf = tr_pool.tile([HG * NBF, TSEG], BF16, tag="rpbf")
            rpe = tr_pool.tile([4, TSEG], F32, tag="rpe")
            vnat = ph2_pool.tile([128, NTILE * GC], BF16, tag="vnat")
            vnatv = vnat[:].rearrange("p (t n) -> p t n", t=NTILE)
            wkbn = ph2_pool.tile([128, NTILE * GC], BF16, tag="wkbn")
            wkbnv = wkbn[:].rearrange("p (t n) -> p t n", t=NTILE)
            wkdwn = ph2_pool.tile([128, NTILE * GC], BF16, tag="wkdwn")
            wkdwnv = wkdwn[:].rearrange("p (t n) -> p t n", t=NTILE)
            gpl = ph2_pool.tile([128, HG * TSEG], BF16, tag="gpl")
            gplv = gpl[:].rearrange("p (h t) -> p h t", h=HG)
            attnT = ph2_pool.tile([128, (HG // 2) * TSEG], BF16, tag="attnT")
            attnTv = attnT[:].rearrange("p (r n) -> p r n", r=HG // 2)
            ATd = ph2_pool.tile([128, HG * TSEG], BF16, tag="ATd")
            ATdv = ATd[:].rearrange("p (h t) -> p h t", h=HG)
            wcdT = ph2_pool.tile([128, HG * TSEG], BF16, tag="wcdT")
            wcdTv = wcdT[:].rearrange("p (h t) -> p h t", h=HG)
            oT = ph2_pool.tile([128, HG * TSEG], BF16, tag="oT")
            oTv = oT[:].rearrange("p (h t) -> p h t", h=HG)
            gam = tr_pool.tile([128, HG * NCHS], F32, tag="gam")

            ph2_q = list(range(NCHS)) if prev is not None else []

            def drip(k=1):
                for _ in range(min(k, len(ph2_q))):
                    emit_ph2_chunk(prev, ph2_q.pop(0))

            # ============ per token-tile: projections + scalar bundle ======
            for tt in range(NTILE):
                psv = ps_a.tile([128, GC], F32, tag="vps")
                pss = ps_d.tile([128, 12], F32, tag="small")
                for kb in range(8):
                    xtt = xTv[:, kb, 1 + tt * 128:1 + (tt + 1) * 128]
                    mm(psv[:], xtt, wcat_v[:, kb, :],
                       start=(kb == 0), stop=(kb == 7))
                for kb in range(8):
                    xtt = xTv[:, kb, 1 + tt * 128:1 + (tt + 1) * 128]
                    mm(pss[:], xtt, wsml_v[:, kb, :],
                       start=(kb == 0), stop=(kb == 7))

                bun = bun_pool.tile([128, HG * NQ], F32, tag="bun")
                bv = bun[:].rearrange("p (h q) -> p h q", h=HG)
                scr = bun_pool.tile([128, 24], F32, tag="scr")
                sq = bun_pool.tile([128, 128], F32, tag="sq")
                # norms -> f (from x) and f_shift (from xs, same pipeline)
                for h in range(HG):
                    nc.scalar.activation(sq[:], xnv[:, tt, h * HD:(h + 1) * HD],
                                         AF.Square, accum_out=scr[:, h:h + 1])
                for h in range(HG):
                    nc.scalar.activation(sq[:], xsv[:, tt, h * HD:(h + 1) * HD],
                                         AF.Square,
                                         accum_out=scr[:, 4 + h:5 + h])
                nc.vector.tensor_scalar_max(scr[:, 8:16], scr[:, 0:8], 1e-24)
                nc.scalar.activation(scr[:, 16:24], scr[:, 8:16], AF.Ln)
                nc.scalar.activation(bv[:, :, QF], scr[:, 16:20], AF.Exp,
                                     scale=-0.5)
                nc.scalar.activation(bv[:, :, QFSH], scr[:, 20:24], AF.Exp,
                                     scale=-0.5)
                # sigmoids
                sg = bun_pool.tile([128, 8], F32, tag="sg")
                nc.scalar.activation(sg[:, 0:4], pss[:, 0:4], AF.Exp,
                                     scale=-1.0)
                nc.scalar.activation(sg[:, 4:8], pss[:, 8:12], AF.Exp,
                                     scale=-1.0)
                nc.vector.tensor_scalar_add(sg[:, 0:8], sg[:, 0:8], 1.0)
                nc.vector.reciprocal(bv[:, :, QBETA], sg[:, 0:4])
                nc.vector.reciprocal(bv[:, :, QG], sg[:, 4:8])
                # decay
                nc.vector.tensor_add(scr[:, 12:16], pss[:, 4:8], dtb_sb[:])
                nc.scalar.activation(scr[:, 16:20], scr[:, 12:16], AF.Exp)
                nc.scalar.activation(scr[:, 16:20], scr[:, 16:20], AF.Ln,
                                     bias=1.0)
                nc.vector.tensor_mul(scr[:, 20:24], scr[:, 16:20], aneg_sb[:])
                # within-chunk cumulative decay
                psc = ps_d.tile([128, 4], F32, tag="small")
                mm(psc[:], triucum_sb[:], scr[:, 20:24])
                nc.scalar.copy(bv[:, :, QDEC], psc[:])
                psl = ps_d.tile([128, 4], F32, tag="small")
                mm(psl[:], e64sel_sb[:], bv[:, :, QDEC])
                nc.vector.tensor_sub(scr[:, 0:4], psl[:], bv[:, :, QDEC])
                nc.scalar.activation(scr[:, 4:8], scr[:, 0:4], AF.Exp)  # dw
                nc.scalar.activation(bv[:, :, QEDEC], bv[:, :, QDEC], AF.Exp)
                nc.scalar.activation(bv[:, :, QEDEC2], bv[:, :, QDEC], AF.Exp)
                nc.scalar.activation(bv[:, :, QEDECI], bv[:, :, QDEC], AF.Exp,
                                     scale=-1.0)
                nc.vector.tensor_mul(bv[:, :, QRA], bv[:, :, QF],
                                     bv[:, :, QEDEC])
                # fold f*edec into the gate: the rk-side per-token factor is
                # applied to pot's output columns via gpl instead of to xT
                nc.vector.tensor_mul(bv[:, :, QG], bv[:, :, QG],
                                     bv[:, :, QRA])
                nc.vector.tensor_mul(scr[:, 8:12], bv[:, :, QFSH],
                                     bv[:, :, QBETA])
                nc.vector.tensor_mul(bv[:, :, QFB], scr[:, 8:12],
                                     bv[:, :, QEDEC])
                nc.vector.tensor_mul(bv[:, :, QWB], bv[:, :, QFSH],
                                     bv[:, :, QEDECI])
                nc.vector.tensor_mul(bv[:, :, QFDW], bv[:, :, QFSH],
                                     scr[:, 4:8])
                # transposed per-token scalars
                bsh = bun_pool.tile([128, HG * NBF], BF16, tag="bsh")
                nc.vector.tensor_copy(
                    bsh[:].rearrange("p (h q) -> p h q", h=HG),
                    bv[:, :, 0:NBF])
                ptb = ps_d.tile([HG * NBF, 128], BF16, tag="small")
                nc.tensor.transpose(ptb[:], bsh[:], identbf_sb[:])
                nc.scalar.copy(rpbf[:, tt * 128:(tt + 1) * 128], ptb[:])
                ptf4 = ps_d.tile([4, 128], F32, tag="small")
                nc.tensor.transpose(ptf4[:], bv[:, :, QEDEC2], identf_sb[:])
                nc.scalar.copy(rpe[:, tt * 128:(tt + 1) * 128], ptf4[:])
                # v' = beta * v  (natural, bf16)
                nc.vector.scalar_tensor_tensor(
                    vnatv[:, tt, :].rearrange("p (h e) -> p h e", h=HG),
                    psv[:].rearrange("p (h e) -> p h e", h=HG), 1.0,
                    bv[:, :, QBETA].broadcast_to((128, HG, HD)),
                    op0=OP.mult, op1=OP.mult)
                # shifted-key natural tensors
                nc.vector.scalar_tensor_tensor(
                    wkbnv[:, tt, :].rearrange("p (h e) -> p h e", h=HG),
                    xsv[:, tt, :].rearrange("p (h e) -> p h e", h=HG), 1.0,
                    bv[:, :, QFB].broadcast_to((128, HG, HD)),
                    op0=OP.mult, op1=OP.mult)
                nc.vector.scalar_tensor_tensor(
                    wkdwnv[:, tt, :].rearrange("p (h e) -> p h e", h=HG),
                    xsv[:, tt, :].rearrange("p (h e) -> p h e", h=HG), 1.0,
                    bv[:, :, QFDW].broadcast_to((128, HG, HD)),
                    op0=OP.mult, op1=OP.mult)
                if tt > 0:
                    drip()

            if stage < 2:
                continue
            # ============ per head: T-side tiles + gamma ============
            wkA_l, wkB_l = [], []
            for h in range(HG):
                wkA = ph1_pool.tile([128, TSEG], BF16, tag=f"wkA{h}")
                wkB = ph1_pool.tile([128, TSEG], BF16, tag=f"wkB{h}")
                wkA_l.append(wkA); wkB_l.append(wkB)

                rp16 = rpbf[0:16, :]
                pa = ps_b.tile([128, TSEG], F32, tag="prod")
                selbf_mm(pa[:], h * NBF + QFB, rp16)
                nc.vector.scalar_tensor_tensor(
                    wkA[:], xTv[:, h, 0:TSEG], 1.0, pa[:],
                    op0=OP.mult, op1=OP.mult)
                pb = ps_b.tile([128, TSEG], F32, tag="prod")
                selbf_mm(pb[:], h * NBF + QWB, rp16)
                nc.vector.scalar_tensor_tensor(
                    wkB[:], xTv[:, h, 0:TSEG], 1.0, pb[:],
                    op0=OP.mult, op1=OP.mult)
                pg = ps_b.tile([128, TSEG], F32, tag="prod")
                selbf_mm(pg[:], h * NBF + QG, rp16)
                nc.scalar.copy(gplv[:, h, :], pg[:])
                # gamma = exp(dec at chunk end): K=1 ones-matmul broadcast
                pgm = ps_d.tile([128, NCHS], F32, tag="small")
                gsel = bass.AP(rpe[:].tensor, rpe[:].offset + 63,
                               [[TSEG, 4], [C, NCHS]])
                mm(pgm[:], sel4_sb[:, h * 128:(h + 1) * 128], gsel)
                nc.scalar.copy(gam[:, h * NCHS:(h + 1) * NCHS], pgm[:])

            if stage < 3:
                continue
            # ===== phase-1: products, masks, 3-factor inverse =====
            # A.T = ((I-M)(I+M^2)(I+M^4)).T, error O(M^8); the two head
            # pairs (pr) are interleaved stage-by-stage so PE never waits
            # on a PSUM drain (the other pair's matmuls cover it).
            def chunk_mms(out_ps, lh, rh, pr):
                for hh in range(2):
                    sl = slice(hh * 64, (hh + 1) * 64)
                    for n in range(NCHS):
                        csl = slice(n * C, (n + 1) * C)
                        mm(out_ps[sl, csl], lh[sl, csl], rh[sl, csl])

            Msb_, MTsb_, ImM_, P1r_, P1i_, Q1r_, G0_, G1_ = ({} for _ in
                                                             range(8))
            for pr in range(2):
                pp1 = ps_b.tile([128, 512], F32, tag="prod")
                for hh in range(2):
                    h = pr * 2 + hh
                    sl = slice(hh * 64, (hh + 1) * 64)
                    for n in range(NCHS):
                        csl = slice(n * C, (n + 1) * C)
                        mm(pp1[sl, csl], wkA_l[h][:, csl], wkB_l[h][:, csl])
                Msb_[pr] = ph1_pool.tile([128, 512], BF16, tag=f"Msb{pr}",
                               name=f"Msb{pr}")
                nc.vector.scalar_tensor_tensor(Msb_[pr][:], pp1[:], 1.0,
                                               mSL_sb[:],
                                               op0=OP.mult, op1=OP.mult)
            for pr in range(2):
                pp1t = ps_b.tile([128, 512], F32, tag="prod")
                for hh in range(2):
                    h = pr * 2 + hh
                    sl = slice(hh * 64, (hh + 1) * 64)
                    for n in range(NCHS):
                        csl = slice(n * C, (n + 1) * C)
                        mm(pp1t[sl, csl], wkB_l[h][:, csl], wkA_l[h][:, csl])
                MTsb_[pr] = ph1_pool.tile([128, 512], BF16, tag=f"MTsb{pr}",
                                name=f"MTsb{pr}")
                nc.vector.scalar_tensor_tensor(MTsb_[pr][:], pp1t[:], 1.0,
                                               mSU_sb[:],
                                               op0=OP.mult, op1=OP.mult)
                ImM_[pr] = ph1_pool.tile([128, 512], BF16, tag=f"ImM{pr}",
                               name=f"ImM{pr}")
                nc.vector.scalar_tensor_tensor(ImM_[pr][:], Msb_[pr][:], -1.0,
                                               i2x8_sb[:],
                                               op0=OP.mult, op1=OP.add)
            drip()
            for pr in range(2):
                pp2 = ps_b.tile([128, 512], F32, tag="prod")
                for hh in range(2):
                    h = pr * 2 + hh
                    sl = slice(hh * 64, (hh + 1) * 64)
                    for n in range(NCHS):
                        csl = slice(n * C, (n + 1) * C)
                        mm(pp2[sl, csl], wkB_l[h][:, csl],
                           xTv[:, h, 1 + n * C:1 + (n + 1) * C])
                nc.vector.scalar_tensor_tensor(attnTv[:, pr, :], pp2[:], 1.0,
                                               mUI_sb[:],
                                               op0=OP.mult, op1=OP.mult)
            drip()
            for pr in range(2):
                pP1 = ps_b.tile([128, 512], F32, tag="prod")
                chunk_mms(pP1, MTsb_[pr][:], Msb_[pr][:], pr)
                P1r_[pr] = ph1_pool.tile([128, 512], BF16, tag=f"P1r{pr}",
                               name=f"P1r{pr}")
                P1i_[pr] = ph1_pool.tile([128, 512], BF16, tag=f"P1i{pr}",
                               name=f"P1i{pr}")
                nc.scalar.copy(P1r_[pr][:], pP1[:])
                nc.vector.scalar_tensor_tensor(P1i_[pr][:], pP1[:], 1.0,
                                               i2x8_sb[:],
                                               op0=OP.mult, op1=OP.add)
            drip()
            for pr in range(2):
                pQ1 = ps_b.tile([128, 512], F32, tag="prod")
                chunk_mms(pQ1, Msb_[pr][:], MTsb_[pr][:], pr)
                Q1r_[pr] = ph1_pool.tile([128, 512], BF16, tag=f"Q1r{pr}",
                               name=f"Q1r{pr}")
                nc.scalar.copy(Q1r_[pr][:], pQ1[:])
            drip()
            P2r_, P2i_, Q2r_, G2_ = {}, {}, {}, {}
            for pr in range(2):
                pP2 = ps_b.tile([128, 512], F32, tag="prod")
                chunk_mms(pP2, Q1r_[pr][:], P1r_[pr][:], pr)
                P2r_[pr] = ph1_pool.tile([128, 512], BF16, tag=f"P2r{pr}",
                                         name=f"P2r{pr}")
                P2i_[pr] = ph1_pool.tile([128, 512], BF16, tag=f"P2i{pr}",
                                         name=f"P2i{pr}")
                nc.scalar.copy(P2r_[pr][:], pP2[:])
                nc.vector.scalar_tensor_tensor(P2i_[pr][:], pP2[:], 1.0,
                                               i2x8_sb[:],
                                               op0=OP.mult, op1=OP.add)
            drip()
            for pr in range(2):
                pQ2 = ps_b.tile([128, 512], F32, tag="prod")
                chunk_mms(pQ2, P1r_[pr][:], Q1r_[pr][:], pr)
                Q2r_[pr] = ph1_pool.tile([128, 512], BF16, tag=f"Q2r{pr}",
                                         name=f"Q2r{pr}")
                nc.scalar.copy(Q2r_[pr][:], pQ2[:])
            for pr in range(2):
                pQ3 = ps_b.tile([128, 512], F32, tag="prod")
                chunk_mms(pQ3, P2r_[pr][:], Q2r_[pr][:], pr)
                G0_[pr] = ph1_pool.tile([128, 512], BF16, tag=f"G0{pr}",
                              name=f"G0{pr}")
                nc.vector.scalar_tensor_tensor(G0_[pr][:], pQ3[:], 1.0,
                                               i2x8_sb[:],
                                               op0=OP.mult, op1=OP.add)
            for pr in range(2):
                pG1 = ps_b.tile([128, 512], F32, tag="prod")
                chunk_mms(pG1, P2i_[pr][:], G0_[pr][:], pr)
                G1_[pr] = ph1_pool.tile([128, 512], BF16, tag=f"G1{pr}",
                              name=f"G1{pr}")
                nc.scalar.copy(G1_[pr][:], pG1[:])
            for pr in range(2):
                pG2 = ps_b.tile([128, 512], F32, tag="prod")
                chunk_mms(pG2, P1i_[pr][:], G1_[pr][:], pr)
                G2_[pr] = ph1_pool.tile([128, 512], BF16, tag=f"G2{pr}",
                                        name=f"G2{pr}")
                nc.scalar.copy(G2_[pr][:], pG2[:])
            for pr in range(2):
                pAT = ps_b.tile([128, 512], F32, tag="prod")
                chunk_mms(pAT, ImM_[pr][:], G2_[pr][:], pr)
                # duplicate each chunk's AT at both partition parities
                for hh in range(2):
                    h = pr * 2 + hh
                    for par in range(2):
                        nc.scalar.copy(
                            ATdv[par * 64:(par + 1) * 64, h, :].rearrange(
                                "p (n c) -> p n c", c=C)[:, par::2, :],
                            pAT[hh * 64:(hh + 1) * 64, :].rearrange(
                                "p (n c) -> p n c", c=C)[:, par::2, :])

            if stage < 4:
                continue
            # wk_cumdecay.T = -(A @ wkb')^T per (head, chunk)
            for h in range(HG):
                pwc = ps_b.tile([128, 512], F32, tag="prod")
                for n in range(NCHS):
                    mm_q(pwc[:, n * C:(n + 1) * C],
                       wkbnv[(n % 2) * 64:(n % 2) * 64 + 64, n // 2,
                             h * HD:(h + 1) * HD],
                       ATdv[(n % 2) * 64:(n % 2) * 64 + 64, h,
                            n * C:(n + 1) * C])
                nc.vector.tensor_scalar_mul(wcdTv[:, h, :], pwc[:], -1.0)

            if stage < 4.5:
                continue
            # flush any chunks of seg s-1 not yet dripped, then its outproj
            drip(NCHS)
            if prev is not None:
                emit_outproj(prev)
            prev = dict(xTv=xTv, vnatv=vnatv, wkbnv=wkbnv, wkdwnv=wkdwnv,
                        attnTv=attnTv, ATdv=ATdv, wcdTv=wcdTv, oTv=oTv,
                        gplv=gplv, gam=gam, t0=t0)

        # epilogue: last segment's recurrence + output projection
        if prev is not None:
            for n in range(NCHS):
                emit_ph2_chunk(prev, n)
            emit_outproj(prev)

    return nc


def _merge_waits(waits):
    """Merge duplicate-sem waits keeping the max threshold (sem-ge modes)."""
    best, order = {}, []
    for w in waits:
        k = getattr(w, "ant_name", None) or str(getattr(w, "id", ""))
        if k not in best:
            best[k] = w
            order.append(k)
        elif (getattr(w, "wait_value", 0) or 0) > (getattr(best[k], "wait_value", 0) or 0):
            best[k] = w
    return [best[k] for k in order]


def _patch_commit_for_wait_caps(tc, nc, cap=1):
    """Wrap TileContext._commit_instruction: instructions whose wait list
    exceeds the ISA sync-slot budget get standalone EventSemaphore carriers
    emitted immediately before them on the same engine."""
    orig = tc._commit_instruction

    def patched(inst, lazy_reg_writes=True):
        si = getattr(inst, "sync_info", None)
        eng = getattr(inst, "engine", None)
        if si is not None and si.on_wait and eng is not None:
            w = _merge_waits(list(si.on_wait))
            if len(w) > cap:
                keep, excess = w[:cap], w[cap:]
                for ww in excess:
                    ev = mybir.InstDrain(
                        name=nc.get_next_instruction_name(),
                        ins=[], outs=[],
                        sync_info=mybir.SyncInfo(on_wait=[ww], on_update=[]))
                    ev.engine = eng
                    orig(ev, lazy_reg_writes=False)
                w = keep
            if len(w) != len(si.on_wait):
                inst.sync_info = mybir.SyncInfo(
                    on_wait=w, on_update=list(si.on_update or []))
        return orig(inst, lazy_reg_writes)

    tc._commit_instruction = patched

    orig_dab = tc._drain_and_barrier

    def patched_dab(tick_clock, wait_clock):
        from concourse.tile import ScopedClock
        d = nc.sync.drain()
        wait_clock.add_sem_waits(
            d.ins, ScopedClock({None: tick_clock.global_clock}))
        si = d.ins.sync_info
        if si is not None and si.on_wait and len(si.on_wait) > 1:
            extra = list(si.on_wait[1:])
            d.ins.sync_info = mybir.SyncInfo(
                on_wait=[si.on_wait[0]],
                on_update=list(si.on_update or []))
            for w in extra:
                d2 = nc.sync.drain()
                d2.ins.sync_info = mybir.SyncInfo(on_wait=[w], on_update=[])
        nc.all_engine_barrier()
        popped = nc._tile_sem_poison_stack.pop()
        assert popped is tc._sem_poison
        nc.clear_and_free_semaphores(list(tc.sems.allocated().values()))
        nc.all_engine_barrier()

    tc._drain_and_barrier = patched_dab


# ---------------- host side ----------------

def _prep_core_inputs(x_b, g, W_write, W_gate, W_out, W_beta, W_alpha,
                      dt_bias, A_log, Ttot):
    perm = np.arange(D) if g == 0 else np.concatenate(
        [np.arange(GC, 2 * GC), np.arange(0, GC)])
    xr = x_b[:, perm]
    hsl = slice(g * HG, (g + 1) * HG)
    Ww = W_write[g * GC:(g + 1) * GC, :][:, perm]
    Wsml = np.concatenate([W_beta[hsl], W_alpha[hsl], W_gate[hsl]], 0)[:, perm]
    Wo = W_out[:, g * GC:(g + 1) * GC]

    wcat_np = np.ascontiguousarray(
        Ww.T.reshape(8, 128, GC).transpose(1, 0, 2)).astype(ml_dtypes.bfloat16)
    wsml_np = np.ascontiguousarray(
        Wsml.T.reshape(8, 128, 12).transpose(1, 0, 2)).astype(ml_dtypes.bfloat16)
    wout_np = np.ascontiguousarray(
        Wo.T.reshape(HG, 128, 1024).transpose(1, 0, 2)).astype(ml_dtypes.bfloat16)
    dtb_np = np.broadcast_to(dt_bias[hsl], (128, HG)).astype(np.float32)
    aneg_np = np.broadcast_to(-np.exp(A_log[hsl]), (128, HG)).astype(np.float32)
    xb = xr[:Ttot].astype(ml_dtypes.bfloat16)
    xthn = np.zeros((8, 128, Ttot + 1), ml_dtypes.bfloat16)
    xthn[:, :, 1:] = np.ascontiguousarray(xb.T).reshape(8, 128, Ttot)
    xnhp = np.zeros((Ttot + 1, GC), ml_dtypes.bfloat16)
    xnhp[1:] = xb[:, 0:GC]
    return {
        "xth": xthn,
        "xnh": xnhp,
        "wcat": wcat_np, "wsml": wsml_np, "wout": wout_np,
        "dtb": np.ascontiguousarray(dtb_np),
        "aneg": np.ascontiguousarray(aneg_np),
    }


_NC_CACHE = {}


def kernel(x, W_write, W_gate, W_out, W_beta, W_alpha, dt_bias, A_log,
           _trace=False):
    from concourse.bass_utils import run_bass_kernel_spmd

    x = np.asarray(x)
    Bn, Tn, Dm = x.shape
    if Tn not in _NC_CACHE:
        _NC_CACHE[Tn] = build_nc(Ttot=Tn)
    nc = _NC_CACHE[Tn]

    in_maps = []
    for core in range(NCORES):
        b, g = core // 2, core % 2
        in_maps.append(_prep_core_inputs(
            np.asarray(x[b]), g, np.asarray(W_write), np.asarray(W_gate),
            np.asarray(W_out), np.asarray(W_beta), np.asarray(W_alpha),
            np.asarray(dt_bias), np.asarray(A_log), Tn))

    res = run_bass_kernel_spmd(nc, in_maps, core_ids=list(range(NCORES)),
                               trace=_trace)
    out = np.empty((Bn, Tn, Dm), np.float32)
    for b in range(Bn):
        p0 = res.results[2 * b]["outp"].reshape(Dm, Tn).astype(np.float32)
        p1 = res.results[2 * b + 1]["outp"].reshape(Dm, Tn).astype(np.float32)
        out[b] = x[b] + p0.T + p1.T
    if _trace:
        kernel._last_results = res
    return out

